# revision 19
# baseline (speedup 1.0000x reference)
"""Trainium2 Bass kernel for nn_Attention_8323646620215.

LayerNorm -> QKV -> scores(+rel-bias+mask) -> softmax -> attn@V -> out proj.

Sharding: 8 cores = (batch b in 0..3) x (query-half in 0..1). Each core
computes the full K/V for its batch and attention for its 1024 query rows;
no cross-core communication.

Fully software-pipelined single-pass schedule:
  - x arrives as bf16 (host-cast), LN on DVE, xn transposed via the DMA
    XBAR (dma_start_transpose) straight into xnT -- no PE transposes.
  - exp(rel-bias)*mask is precomputed on the host and DMA'd as bf16
    [N, NQ] tiles (expA); the mask enters multiplicatively:
    P = exp(S) * expA.
  - QKV production (PE matmuls + psum->SBUF copies) is interleaved into
    the attention head-pair passes so the PE never drains.
  - Per head-pair pass: S_T = K_T^T Q_T (two 64-row PE tiles), ACT exp,
    DVE/Pool multiply by expA, AV accumulate with an augmented ones-row
    for the softmax denominators.
  - Denominators: broadcast-DMA the den row across 64 partitions, then a
    single tensor_tensor divide per head (no DRAM round trip).
  - Output projection immediately follows the last pass.

LN gamma / attention scale / beta are folded into the QKV weights
host-side. The host permutes each core's tokens so its own query half
comes first, keeping the device program identical across cores.
"""
import sys
import types
import numpy as np

sys.path.insert(0, "/opt/trn_rl_repo")

# ---- environment fixes (axon agent container) -------------------------------
if "antenv.axon_hooks" not in sys.modules:
    _m = types.ModuleType("antenv.axon_hooks")
    _m._hook = None
    _m.set_axon_ntff_profile_hook = lambda h: setattr(_m, "_hook", h)
    _m.get_axon_ntff_profile_hook = lambda: _m._hook
    sys.modules["antenv.axon_hooks"] = _m
    try:
        from trn_agent_boot.trn_boot import _ntff_profile_via_ctypes
        _m._hook = _ntff_profile_via_ctypes("/opt/axon/libaxon_pjrt.so")
    except Exception:
        pass

import ml_dtypes  # noqa: E402
from concourse import bass, mybir, tile  # noqa: E402
from concourse.bass_utils import run_bass_kernel_spmd  # noqa: E402

F32 = mybir.dt.float32
BF16 = mybir.dt.bfloat16
AF = mybir.ActivationFunctionType
OP = mybir.AluOpType

B, N, D, H, DH, MAXREL = 4, 2048, 512, 8, 64, 200
NQ = N // 2          # queries per core
NT = N // 128        # 16 token tiles
NCORES = 8

# This container's walrus rejects instructions with more than one sem wait.
# Splitting is sound: a same-engine NoOp right before the instruction
# enforces the wait at the same program point (sequencers run in order).


def _split_waits(nc, maxw=1):
    n_split = 0
    for f in nc.m.functions:
        for blk in f.blocks:
            bb = blk.bb if hasattr(blk, "bb") else blk
            insts = list(bb.instructions)
            out = []
            changed = False
            for inst in insts:
                si = inst.sync_info
                waits = list(si.on_wait) if si and si.on_wait else []
                if len(waits) > maxw:
                    extra = waits[:-maxw]
                    chunks = [extra[j:j + maxw] for j in range(0, len(extra), maxw)]
                    for i, chunk in enumerate(chunks):
                        nop = mybir.InstNoOp(name=f"{inst.name}-ws{i}", ins=[], outs=[])
                        nop.engine = inst.engine
                        nop.sync_info = mybir.SyncInfo(on_wait=chunk, on_update=[])
                        out.append(nop)
                    si.on_wait = waits[-maxw:]
                    changed = True
                    n_split += 1
                out.append(inst)
            if changed:
                bb.instructions = out
    return n_split


def build(has_c=False, has_b=False, split=True, dbg=False):
    nc = bass.Bass("TRN2", target_bir_lowering=False, debug=False,
                   num_devices=NCORES)
    x_d = nc.dram_tensor("x", [N, D], BF16, kind="ExternalInput")
    wqkv_d = nc.dram_tensor("wqkv", [D, 3 * D], BF16, kind="ExternalInput")
    cqkv_d = nc.dram_tensor("cqkv", [3 * D], F32, kind="ExternalInput")
    wout_d = nc.dram_tensor("wout", [D, D], BF16, kind="ExternalInput")
    bout_d = nc.dram_tensor("bout", [D], F32, kind="ExternalInput")
    expa_d = nc.dram_tensor("expa", [N, NQ], BF16, kind="ExternalInput")
    dsb_d = nc.dram_tensor("den_scratch", [H, NQ], BF16)
    dsi_d = nc.dram_tensor("invden_scratch", [H, NQ], BF16)
    y_d = nc.dram_tensor("y", [NQ, D], F32, kind="ExternalOutput")

    with tile.TileContext(nc) as tc, \
         tc.tile_pool(name="const", bufs=1) as C, \
         tc.tile_pool(name="pers", bufs=1) as P, \
         tc.tile_pool(name="work", bufs=3) as W:

        # ---- persistent tiles ----------------------------------------------
        xnT = P.tile([128, 4, N], BF16, tag="xnT", name="xnT")
        KTp = [P.tile([128, N], BF16, tag=f"KT{hp}", name=f"KT{hp}") for hp in range(4)]
        QTp = [P.tile([128, NQ], BF16, tag=f"QT{hp}", name=f"QT{hp}") for hp in range(4)]
        Vau = [P.tile([128, H, 66], BF16, tag=f"V{t}", name=f"Vau{t}") for t in range(NT)]
        expA = [P.tile([128, NQ], BF16, tag=f"eA{t}", name=f"eA{t}") for t in range(NT)]
        numT = [P.tile([65, NQ], BF16, tag=f"nT{h}", name=f"nT{h}") for h in range(H)]
        pairT = [P.tile([128, NQ], BF16, tag=f"pT{hp}", name=f"pT{hp}") for hp in range(4)]

        wqkv_sb = [C.tile([128, 3 * D], BF16, tag=f"wqkv{kb}", name=f"wqkv{kb}") for kb in range(4)]
        woutP = [C.tile([128, D], BF16, tag=f"woutP{hp}", name=f"woutP{hp}") for hp in range(4)]

        # ---- input DMAs -----------------------------------------------------
        # sync (SP HWDGE): x tiles then expA tiles (+ later den broadcasts, y).
        # scalar (ACT HWDGE): XBAR transposes of xn (issued inline in LN flow).
        # gpsimd (SWDGE): weights + small constants.
        x_ts = [W.tile([128, D], BF16, tag="x", bufs=6, name=f"x{t}") for t in range(NT)]
        for t in range(NT):
            nc.sync.dma_start(out=x_ts[t][:], in_=x_d[t * 128:(t + 1) * 128, :])
        for kb in range(4):
            nc.gpsimd.dma_start(out=wqkv_sb[kb][:],
                                in_=wqkv_d[kb * 128:(kb + 1) * 128, :])
        for t in range(NT):
            nc.sync.dma_start(out=expA[t][:],
                              in_=expa_d[t * 128:(t + 1) * 128, :])
        for hp in range(4):
            nc.gpsimd.dma_start(out=woutP[hp][:],
                                in_=wout_d[hp * 128:(hp + 1) * 128, :])
        cq_all = C.tile([128, 12], F32, tag="cq")
        cv_bc = C.tile([128, D], F32, tag="cv")
        bout_bc = C.tile([128, D], F32, tag="bout")
        if has_c:
            nc.gpsimd.dma_start(
                out=cq_all[:],
                in_=bass.AP(tensor=cqkv_d.ap().tensor, offset=0,
                            ap=[[1, 128], [128, 12]]))
            nc.gpsimd.dma_start(
                out=cv_bc[:],
                in_=bass.AP(tensor=cqkv_d.ap().tensor, offset=2 * D,
                            ap=[[0, 128], [1, D]]))
        if has_b:
            nc.gpsimd.dma_start(
                out=bout_bc[:],
                in_=bass.AP(tensor=bout_d.ap().tensor, offset=0,
                            ap=[[0, 128], [1, D]]))

        eps_t = C.tile([128, 1], F32, tag="eps")
        nc.vector.memset(eps_t[:], 1e-5)
        for t in range(NT):
            nc.gpsimd.memset(Vau[t][:, :, 64:66], 1.0)

        with tc.tile_pool(name="ps", bufs=2, space="PSUM") as ps:
            # ---- LayerNorm + XBAR transpose (prologue, DVE+ACT) ------------
            def ln_tile(t):
                st = W.tile([128, 6], F32, tag="st")
                nc.vector.bn_stats(out=st[:], in_=x_ts[t][:])
                mv = W.tile([128, 2], F32, tag="mv", bufs=4)
                nc.vector.bn_aggr(out=mv[:], in_=st[:])
                rs = W.tile([128, 1], F32, tag="rs", bufs=4)
                nc.scalar.activation(out=rs[:], in_=mv[:, 1:2], func=AF.Sqrt,
                                     bias=eps_t[:])
                nc.vector.reciprocal(out=rs[:], in_=rs[:])
                xn = W.tile([128, D], BF16, tag="xn", bufs=3)
                nc.gpsimd.tensor_scalar(out=xn[:], in0=x_ts[t][:],
                                        scalar1=mv[:, 0:1],
                                        scalar2=rs[:],
                                        op0=OP.subtract, op1=OP.mult)
                nc.scalar.dma_start_transpose(
                    xnT[:, :, t * 128:(t + 1) * 128], xn[:])

            # ---- QKV production tasks (PE + copy), interleaved -------------
            def q_task(hp):
                def f():
                    qp = ps.tile([128, NQ], F32, tag="sp")
                    for ic in range(2):
                        for kb in range(4):
                            nc.tensor.matmul(
                                qp[:, ic * 512:(ic + 1) * 512],
                                wqkv_sb[kb][:, hp * 128:(hp + 1) * 128],
                                xnT[:, kb, ic * 512:(ic + 1) * 512],
                                start=(kb == 0), stop=(kb == 3))
                    if has_c:
                        nc.vector.tensor_scalar_add(
                            out=QTp[hp][:], in0=qp[:],
                            scalar1=cq_all[:, hp:hp + 1])
                    else:
                        nc.scalar.copy(out=QTp[hp][:], in_=qp[:])
                return f

            def k_task(hp, half):
                def f():
                    kp = ps.tile([128, NQ], F32, tag="sp")
                    for tc4 in range(2):
                        for kb in range(4):
                            nc.tensor.matmul(
                                kp[:, tc4 * 512:(tc4 + 1) * 512],
                                wqkv_sb[kb][:, D + hp * 128:D + (hp + 1) * 128],
                                xnT[:, kb,
                                    half * 1024 + tc4 * 512:
                                    half * 1024 + (tc4 + 1) * 512],
                                start=(kb == 0), stop=(kb == 3))
                    if has_c:
                        nc.vector.tensor_scalar_add(
                            out=KTp[hp][:, half * 1024:(half + 1) * 1024],
                            in0=kp[:], scalar1=cq_all[:, 4 + hp:5 + hp])
                    else:
                        nc.scalar.copy(
                            out=KTp[hp][:, half * 1024:(half + 1) * 1024],
                            in_=kp[:])
                return f

            def v_task(j2):
                def f():
                    vp = ps.tile([128, NQ], F32, tag="sp")
                    for e in range(2):
                        jt = 2 * j2 + e
                        for kb in range(4):
                            nc.tensor.matmul(
                                vp[:, e * 512:(e + 1) * 512],
                                xnT[:, kb, jt * 128:(jt + 1) * 128],
                                wqkv_sb[kb][:, 2 * D:3 * D],
                                start=(kb == 0), stop=(kb == 3))
                    for e in range(2):
                        jt = 2 * j2 + e
                        src = vp[:, e * 512:(e + 1) * 512].rearrange(
                            "p (h d) -> p h d", h=H)
                        if has_c:
                            nc.vector.tensor_add(
                                out=Vau[jt][:, :, 0:64], in0=src,
                                in1=cv_bc[:].rearrange("p (h d) -> p h d", h=H))
                        else:
                            nc.scalar.copy(out=Vau[jt][:, :, 0:64], in_=src)
                return f

            # Emit LN for the query half, then enough QKV to start attention.
            for t in range(4):
                ln_tile(t)
            for t in range(4, 8):
                ln_tile(t)
            q_task(0)()
            k_task(0, 0)()
            for t in range(8, 12):
                ln_tile(t)
            v_task(0)()
            v_task(1)()
            for t in range(12, 16):
                ln_tile(t)

            # prod tasks popped one per jt during passes; V(j2) must land
            # before jt=2*j2 of hp0, K/Q for pass hp before that pass starts.
            prod_by_hp = {
                0: [v_task(2), v_task(3), v_task(4), v_task(5), k_task(0, 1),
                    v_task(6), v_task(7), q_task(1), k_task(1, 0),
                    k_task(1, 1)],
                1: [q_task(2), k_task(2, 0), k_task(2, 1)],
                2: [q_task(3), k_task(3, 0), k_task(3, 1)],
                3: [],
            }

            # ---- attention passes ------------------------------------------
            av0 = ps.tile([65, NQ], F32, tag="av0", bufs=1, name="av0")
            av1 = ps.tile([65, NQ], F32, tag="av1", bufs=1, name="av1")

            def den_pieces(hp):
                """Normalize pair hp's heads; interleaved into next pass.
                Reciprocal runs in place on the single den row; one DRAM
                bounce broadcasts 1/den across 64 partitions."""
                h0, h1 = 2 * hp, 2 * hp + 1
                dbs = [None, None]

                def recip(e):
                    def f():
                        h = 2 * hp + e
                        with nc.allow_low_precision(reason="bf16 softmax denominators"):
                            nc.vector.reciprocal(out=numT[h][64:65, :],
                                                 in_=numT[h][64:65, :])
                        nc.sync.dma_start(out=dsb_d[h, :],
                                          in_=numT[h][64:65, :])
                    return f

                def load_bc(e):
                    def f():
                        h = 2 * hp + e
                        den_bc = W.tile([64, NQ], BF16, tag="denb", bufs=2,
                                        name=f"denb{h}")
                        dbs[e] = den_bc
                        nc.sync.dma_start(
                            out=den_bc[:],
                            in_=bass.AP(tensor=dsb_d.ap().tensor, offset=h * NQ,
                                        ap=[[0, 64], [1, NQ]]))
                    return f

                def mul_half(e, half):
                    def f():
                        sl = slice(half * 512, (half + 1) * 512)
                        if e == 0:
                            nc.vector.tensor_mul(out=pairT[hp][0:64, sl],
                                                 in0=numT[h0][0:64, sl],
                                                 in1=dbs[0][:, sl])
                        else:
                            nc.vector.tensor_mul(out=numT[h1][0:64, sl],
                                                 in0=numT[h1][0:64, sl],
                                                 in1=dbs[1][:, sl])
                    return f

                def stitch():
                    nc.sync.dma_start(out=pairT[hp][64:128, :],
                                      in_=numT[h1][0:64, :])

                return [recip(0), recip(1), load_bc(0), load_bc(1),
                        mul_half(0, 0), mul_half(0, 1),
                        mul_half(1, 0), mul_half(1, 1), stitch]

            pend = []
            it_idx = 0
            for hp in range(4):
                h0, h1 = 2 * hp, 2 * hp + 1
                for jt in range(NT):
                    for ic in range(2):
                        i5 = ic * 512
                        sp = ps.tile([128, NQ], F32, tag="sp")
                        nc.tensor.matmul(
                            sp[:, 0:512],
                            KTp[hp][0:64, jt * 128:(jt + 1) * 128],
                            QTp[hp][0:64, i5:i5 + 512],
                            start=True, stop=True, tile_position=(0, 0))
                        nc.tensor.matmul(
                            sp[:, 512:1024],
                            KTp[hp][64:128, jt * 128:(jt + 1) * 128],
                            QTp[hp][64:128, i5:i5 + 512],
                            start=True, stop=True, tile_position=(64, 0))
                        eb = W.tile([128, NQ], BF16, tag="eb", bufs=4)
                        nc.scalar.activation(out=eb[:], in_=sp[:], func=AF.Exp)
                        pb = W.tile([128, NQ], BF16, tag="pb", bufs=4)
                        ea = expA[jt][:, i5:i5 + 512].unsqueeze(1) \
                            .broadcast_to([128, 2, 512])
                        nc.vector.tensor_tensor(
                            out=pb[:].rearrange("p (a b) -> p a b", a=2),
                            in0=eb[:].rearrange("p (a b) -> p a b", a=2),
                            in1=ea, op=OP.mult)
                        it_idx += 1
                        nc.tensor.matmul(av0[:, i5:i5 + 512],
                                         Vau[jt][:, h0, 0:65], pb[:, 0:512],
                                         start=(jt == 0), stop=(jt == NT - 1))
                        nc.tensor.matmul(av1[:, i5:i5 + 512],
                                         Vau[jt][:, h1, 0:65], pb[:, 512:1024],
                                         start=(jt == 0), stop=(jt == NT - 1))
                    if prod_by_hp[hp]:
                        prod_by_hp[hp].pop(0)()
                    while pend:
                        f = pend.pop(0)
                        if f is not None:
                            f()
                            break
                for f in pend:
                    if f is not None:
                        f()
                nc.vector.tensor_copy(out=numT[h0][:], in_=av0[:])
                nc.scalar.copy(out=numT[h1][:], in_=av1[:])
                pend = den_pieces(hp)
            for f in pend:
                if f is not None:
                    f()

        if dbg:
            dbg_xnT = nc.dram_tensor("dbg_xnT", [128, 4 * N], BF16,
                                     kind="ExternalOutput")
            nc.sync.dma_start(out=dbg_xnT[:, :],
                              in_=xnT[:].rearrange("p a b -> p (a b)"))
            dbg_QT = nc.dram_tensor("dbg_QT", [128, NQ], BF16,
                                    kind="ExternalOutput")
            nc.sync.dma_start(out=dbg_QT[:, :], in_=QTp[0][:])
            dbg_KT = nc.dram_tensor("dbg_KT", [128, N], BF16,
                                    kind="ExternalOutput")
            nc.sync.dma_start(out=dbg_KT[:, :], in_=KTp[0][:])
            dbg_V = nc.dram_tensor("dbg_V", [128, H * 66], BF16,
                                   kind="ExternalOutput")
            nc.sync.dma_start(out=dbg_V[:, :],
                              in_=Vau[0][:].rearrange("p a b -> p (a b)"))
            dbg_nT = nc.dram_tensor("dbg_nT", [65, NQ], BF16,
                                    kind="ExternalOutput")
            nc.sync.dma_start(out=dbg_nT[:, :], in_=numT[0][:])
            dbg_pT = nc.dram_tensor("dbg_pT", [128, NQ], BF16,
                                    kind="ExternalOutput")
            nc.sync.dma_start(out=dbg_pT[:, :], in_=pairT[0][:])

        # ---- output projection ---------------------------------------------
        with tc.tile_pool(name="psD", bufs=1, space="PSUM") as psD:
            for isl in range(8):
                yp = psD.tile([128, D], F32, tag="yp", bufs=4, name=f"yp{isl}")
                for hp in range(4):
                    nc.tensor.matmul(yp[:],
                                     pairT[hp][:, isl * 128:(isl + 1) * 128],
                                     woutP[hp][:],
                                     start=(hp == 0), stop=(hp == 3))
                ysb = W.tile([128, D], F32, tag="ysb", bufs=4)
                if has_b:
                    nc.vector.tensor_add(out=ysb[:], in0=yp[:], in1=bout_bc[:])
                elif isl % 2 == 0:
                    nc.vector.tensor_copy(out=ysb[:], in_=yp[:])
                else:
                    nc.scalar.copy(out=ysb[:], in_=yp[:])
                eng = nc.sync if isl % 2 == 0 else nc.scalar
                eng.dma_start(out=y_d[isl * 128:(isl + 1) * 128, :],
                              in_=ysb[:])
    if split:
        _split_waits(nc)
    return nc


_NC_CACHE = {}


def _get_nc(has_c, has_b):
    key = (has_c, has_b)
    if key not in _NC_CACHE:
        _NC_CACHE[key] = build(has_c, has_b)
    return _NC_CACHE[key]


LAST_EXEC_TIME_NS = None


def kernel(x, gamma, beta, Wqkv, Wout, bout, rel_table, temporal_mask,
           trace=True):
    global LAST_EXEC_TIME_NS
    x = np.asarray(x, np.float32)
    gamma = np.asarray(gamma, np.float32)
    beta = np.asarray(beta, np.float32)
    Wqkv = np.asarray(Wqkv, np.float32)
    Wout = np.asarray(Wout, np.float32)
    bout = np.asarray(bout, np.float32)
    rel_table = np.asarray(rel_table, np.float32)
    temporal_mask = np.asarray(temporal_mask)

    scale = DH ** -0.5
    w_eff = (Wqkv * gamma[:, None]).copy()
    w_eff[:, :D] *= scale
    cqkv = (beta @ Wqkv).astype(np.float32)
    cqkv[:D] *= scale
    wqkv_bf = w_eff.astype(ml_dtypes.bfloat16)
    wout_bf = Wout.astype(ml_dtypes.bfloat16)
    mask01T = (temporal_mask != 0).T  # [key j, query i]

    exp_table = np.exp(rel_table).astype(np.float32)
    keyperm_half = [
        np.concatenate([np.arange(i0, i0 + NQ),
                        np.arange(NQ - i0, NQ - i0 + NQ)])
        for i0 in (0, NQ)
    ]
    expa_half = []
    for half in range(2):
        kp = keyperm_half[half]
        d = (half * NQ + np.arange(NQ))[None, :] - kp[:, None]
        idx = np.clip(d, -(MAXREL - 1), MAXREL - 1) + MAXREL - 1
        ea = exp_table[idx] * mask01T[kp][:, half * NQ:(half + 1) * NQ]
        expa_half.append(ea.astype(ml_dtypes.bfloat16))

    in_maps = []
    for c in range(NCORES):
        b, half = c // 2, c % 2
        xp = np.ascontiguousarray(
            x[b][keyperm_half[half]]).astype(ml_dtypes.bfloat16)
        in_maps.append({
            "x": xp,
            "wqkv": wqkv_bf,
            "cqkv": cqkv,
            "wout": wout_bf,
            "bout": bout,
            "expa": expa_half[half],
        })

    nc = _get_nc(bool(np.any(cqkv != 0.0)), bool(np.any(bout != 0.0)))
    res = run_bass_kernel_spmd(nc, in_maps, core_ids=list(range(NCORES)),
                               trace=trace)
    LAST_EXEC_TIME_NS = res.exec_time_ns

    out = np.empty((B, N, D), np.float32)
    for c in range(NCORES):
        b, half = c // 2, c % 2
        out[b, half * NQ:(half + 1) * NQ] = res.results[c]["y"]
    return out


# revision 20
# speedup vs baseline: 1.1929x; 1.1929x over previous
"""Trainium2 Bass kernel for nn_Attention_8323646620215.

LayerNorm -> QKV -> scores(+rel-bias+mask) -> softmax -> attn@V -> out proj.

Sharding: 8 cores = (batch b in 0..3) x (query-half in 0..1). Each core
computes the full K/V for its batch and attention for its 1024 query rows;
no cross-core communication.

Fully software-pipelined single-pass schedule:
  - x arrives as bf16 (host-cast), LN on DVE, xn transposed via the DMA
    XBAR (dma_start_transpose) straight into xnT -- no PE transposes.
  - exp(rel-bias)*mask is precomputed on the host and DMA'd as bf16
    [N, NQ] tiles (expA); the mask enters multiplicatively:
    P = exp(S) * expA.
  - QKV production (PE matmuls + psum->SBUF copies) is interleaved into
    the attention head-pair passes so the PE never drains.
  - Per head-pair pass: S_T = K_T^T Q_T (two 64-row PE tiles), ACT exp,
    DVE/Pool multiply by expA, AV accumulate with an augmented ones-row
    for the softmax denominators.
  - Denominators: broadcast-DMA the den row across 64 partitions, then a
    single tensor_tensor divide per head (no DRAM round trip).
  - Output projection immediately follows the last pass.

LN gamma / attention scale / beta are folded into the QKV weights
host-side. The host permutes each core's tokens so its own query half
comes first, keeping the device program identical across cores.
"""
import sys
import types
import numpy as np

sys.path.insert(0, "/opt/trn_rl_repo")

# ---- environment fixes (axon agent container) -------------------------------
if "antenv.axon_hooks" not in sys.modules:
    _m = types.ModuleType("antenv.axon_hooks")
    _m._hook = None
    _m.set_axon_ntff_profile_hook = lambda h: setattr(_m, "_hook", h)
    _m.get_axon_ntff_profile_hook = lambda: _m._hook
    sys.modules["antenv.axon_hooks"] = _m
    try:
        from trn_agent_boot.trn_boot import _ntff_profile_via_ctypes
        _m._hook = _ntff_profile_via_ctypes("/opt/axon/libaxon_pjrt.so")
    except Exception:
        pass

import ml_dtypes  # noqa: E402
from concourse import bass, mybir, tile  # noqa: E402
from concourse.bass_utils import run_bass_kernel_spmd  # noqa: E402

F32 = mybir.dt.float32
BF16 = mybir.dt.bfloat16
AF = mybir.ActivationFunctionType
OP = mybir.AluOpType

B, N, D, H, DH, MAXREL = 4, 2048, 512, 8, 64, 200
NQ = N // 2          # queries per core
NT = N // 128        # 16 token tiles
NCORES = 8

# This container's walrus rejects instructions with more than one sem wait.
# Splitting is sound: a same-engine NoOp right before the instruction
# enforces the wait at the same program point (sequencers run in order).


def _split_waits(nc, maxw=1):
    n_split = 0
    for f in nc.m.functions:
        for blk in f.blocks:
            bb = blk.bb if hasattr(blk, "bb") else blk
            insts = list(bb.instructions)
            out = []
            changed = False
            for inst in insts:
                si = inst.sync_info
                waits = list(si.on_wait) if si and si.on_wait else []
                if len(waits) > maxw:
                    extra = waits[:-maxw]
                    chunks = [extra[j:j + maxw] for j in range(0, len(extra), maxw)]
                    for i, chunk in enumerate(chunks):
                        nop = mybir.InstNoOp(name=f"{inst.name}-ws{i}", ins=[], outs=[])
                        nop.engine = inst.engine
                        nop.sync_info = mybir.SyncInfo(on_wait=chunk, on_update=[])
                        out.append(nop)
                    si.on_wait = waits[-maxw:]
                    changed = True
                    n_split += 1
                out.append(inst)
            if changed:
                bb.instructions = out
    return n_split


def build(has_c=False, has_b=False, split=True, dbg=False):
    nc = bass.Bass("TRN2", target_bir_lowering=False, debug=False,
                   num_devices=NCORES)
    x_d = nc.dram_tensor("x", [N, D], BF16, kind="ExternalInput")
    wqkv_d = nc.dram_tensor("wqkv", [D, 3 * D], BF16, kind="ExternalInput")
    cqkv_d = nc.dram_tensor("cqkv", [3 * D], F32, kind="ExternalInput")
    wout_d = nc.dram_tensor("wout", [D, D], BF16, kind="ExternalInput")
    bout_d = nc.dram_tensor("bout", [D], F32, kind="ExternalInput")
    expa_d = nc.dram_tensor("expa", [N, NQ], BF16, kind="ExternalInput")
    dsb_d = nc.dram_tensor("den_scratch", [H, NQ], BF16)
    dsi_d = nc.dram_tensor("invden_scratch", [H, NQ], BF16)
    y_d = nc.dram_tensor("y", [NQ, D], F32, kind="ExternalOutput")

    with tile.TileContext(nc) as tc, \
         tc.tile_pool(name="const", bufs=1) as C, \
         tc.tile_pool(name="pers", bufs=1) as P, \
         tc.tile_pool(name="work", bufs=3) as W:

        # ---- persistent tiles ----------------------------------------------
        xnT = P.tile([128, 4, N], BF16, tag="xnT", name="xnT")
        KTp = [P.tile([128, N], BF16, tag=f"KT{hp}", name=f"KT{hp}") for hp in range(4)]
        QTp = [P.tile([128, NQ], BF16, tag=f"QT{hp}", name=f"QT{hp}") for hp in range(4)]
        Vau = [P.tile([128, H, 66], BF16, tag=f"V{t}", name=f"Vau{t}") for t in range(NT)]
        expA = [P.tile([128, NQ], BF16, tag=f"eA{t}", name=f"eA{t}") for t in range(NT)]
        numT = [P.tile([65, NQ], BF16, tag=f"nT{h}", name=f"nT{h}") for h in range(H)]
        pairT = [P.tile([128, NQ], BF16, tag=f"pT{hp}", name=f"pT{hp}") for hp in range(4)]

        wqkv_sb = [C.tile([128, 3 * D], BF16, tag=f"wqkv{kb}", name=f"wqkv{kb}") for kb in range(4)]
        woutP = [C.tile([128, D], BF16, tag=f"woutP{hp}", name=f"woutP{hp}") for hp in range(4)]

        # ---- input DMAs -----------------------------------------------------
        # sync (SP HWDGE): x tiles then expA tiles (+ later den broadcasts, y).
        # scalar (ACT HWDGE): XBAR transposes of xn (issued inline in LN flow).
        # gpsimd (SWDGE): weights + small constants.
        x_ts = [W.tile([128, D], BF16, tag="x", bufs=6, name=f"x{t}") for t in range(NT)]
        for t in range(NT):
            nc.sync.dma_start(out=x_ts[t][:], in_=x_d[t * 128:(t + 1) * 128, :])
        for kb in range(4):
            nc.gpsimd.dma_start(out=wqkv_sb[kb][:],
                                in_=wqkv_d[kb * 128:(kb + 1) * 128, :])
        for t in range(NT):
            nc.sync.dma_start(out=expA[t][:],
                              in_=expa_d[t * 128:(t + 1) * 128, :])
        for hp in range(4):
            nc.gpsimd.dma_start(out=woutP[hp][:],
                                in_=wout_d[hp * 128:(hp + 1) * 128, :])
        cq_all = C.tile([128, 12], F32, tag="cq")
        cv_bc = C.tile([128, D], F32, tag="cv")
        bout_bc = C.tile([128, D], F32, tag="bout")
        if has_c:
            nc.gpsimd.dma_start(
                out=cq_all[:],
                in_=bass.AP(tensor=cqkv_d.ap().tensor, offset=0,
                            ap=[[1, 128], [128, 12]]))
            nc.gpsimd.dma_start(
                out=cv_bc[:],
                in_=bass.AP(tensor=cqkv_d.ap().tensor, offset=2 * D,
                            ap=[[0, 128], [1, D]]))
        if has_b:
            nc.gpsimd.dma_start(
                out=bout_bc[:],
                in_=bass.AP(tensor=bout_d.ap().tensor, offset=0,
                            ap=[[0, 128], [1, D]]))

        eps_t = C.tile([128, 1], F32, tag="eps")
        nc.vector.memset(eps_t[:], 1e-5)
        for t in range(NT):
            nc.gpsimd.memset(Vau[t][:, :, 64:66], 1.0)

        with tc.tile_pool(name="ps", bufs=2, space="PSUM") as ps:
            # ---- LayerNorm + XBAR transpose (prologue, DVE+ACT) ------------
            def ln_tile(t):
                st = W.tile([128, 6], F32, tag="st")
                nc.vector.bn_stats(out=st[:], in_=x_ts[t][:])
                mv = W.tile([128, 2], F32, tag="mv", bufs=4)
                nc.vector.bn_aggr(out=mv[:], in_=st[:])
                rs = W.tile([128, 1], F32, tag="rs", bufs=4)
                nc.scalar.activation(out=rs[:], in_=mv[:, 1:2], func=AF.Sqrt,
                                     bias=eps_t[:])
                nc.vector.reciprocal(out=rs[:], in_=rs[:])
                xn = W.tile([128, D], BF16, tag="xn", bufs=3)
                nc.vector.tensor_scalar(out=xn[:], in0=x_ts[t][:],
                                        scalar1=mv[:, 0:1],
                                        scalar2=rs[:],
                                        op0=OP.subtract, op1=OP.mult)
                nc.scalar.dma_start_transpose(
                    xnT[:, :, t * 128:(t + 1) * 128], xn[:])

            # ---- QKV production tasks (PE + copy), interleaved -------------
            def q_task(hp):
                def f():
                    qp = ps.tile([128, NQ], F32, tag="sp")
                    for ic in range(2):
                        for kb in range(4):
                            nc.tensor.matmul(
                                qp[:, ic * 512:(ic + 1) * 512],
                                wqkv_sb[kb][:, hp * 128:(hp + 1) * 128],
                                xnT[:, kb, ic * 512:(ic + 1) * 512],
                                start=(kb == 0), stop=(kb == 3))
                    if has_c:
                        nc.vector.tensor_scalar_add(
                            out=QTp[hp][:], in0=qp[:],
                            scalar1=cq_all[:, hp:hp + 1])
                    else:
                        nc.scalar.copy(out=QTp[hp][:], in_=qp[:])
                return f

            def k_task(hp, half):
                def f():
                    kp = ps.tile([128, NQ], F32, tag="sp")
                    for tc4 in range(2):
                        for kb in range(4):
                            nc.tensor.matmul(
                                kp[:, tc4 * 512:(tc4 + 1) * 512],
                                wqkv_sb[kb][:, D + hp * 128:D + (hp + 1) * 128],
                                xnT[:, kb,
                                    half * 1024 + tc4 * 512:
                                    half * 1024 + (tc4 + 1) * 512],
                                start=(kb == 0), stop=(kb == 3))
                    if has_c:
                        nc.vector.tensor_scalar_add(
                            out=KTp[hp][:, half * 1024:(half + 1) * 1024],
                            in0=kp[:], scalar1=cq_all[:, 4 + hp:5 + hp])
                    else:
                        nc.scalar.copy(
                            out=KTp[hp][:, half * 1024:(half + 1) * 1024],
                            in_=kp[:])
                return f

            def v_task(j2):
                def f():
                    vp = ps.tile([128, NQ], F32, tag="sp")
                    for e in range(2):
                        jt = 2 * j2 + e
                        for kb in range(4):
                            nc.tensor.matmul(
                                vp[:, e * 512:(e + 1) * 512],
                                xnT[:, kb, jt * 128:(jt + 1) * 128],
                                wqkv_sb[kb][:, 2 * D:3 * D],
                                start=(kb == 0), stop=(kb == 3))
                    for e in range(2):
                        jt = 2 * j2 + e
                        src = vp[:, e * 512:(e + 1) * 512].rearrange(
                            "p (h d) -> p h d", h=H)
                        if has_c:
                            nc.vector.tensor_add(
                                out=Vau[jt][:, :, 0:64], in0=src,
                                in1=cv_bc[:].rearrange("p (h d) -> p h d", h=H))
                        else:
                            nc.scalar.copy(out=Vau[jt][:, :, 0:64], in_=src)
                return f

            # Emit LN for the query half, then enough QKV to start attention.
            for t in range(4):
                ln_tile(t)
            for t in range(4, 8):
                ln_tile(t)
            q_task(0)()
            k_task(0, 0)()
            for t in range(8, 12):
                ln_tile(t)
            v_task(0)()
            v_task(1)()
            for t in range(12, 16):
                ln_tile(t)

            # prod tasks popped one per jt during passes; V(j2) must land
            # before jt=2*j2 of hp0, K/Q for pass hp before that pass starts.
            prod_by_hp = {
                0: [v_task(2), v_task(3), v_task(4), v_task(5), k_task(0, 1),
                    v_task(6), v_task(7), q_task(1), k_task(1, 0),
                    k_task(1, 1)],
                1: [q_task(2), k_task(2, 0), k_task(2, 1)],
                2: [q_task(3), k_task(3, 0), k_task(3, 1)],
                3: [],
            }

            # ---- attention passes ------------------------------------------
            av0 = ps.tile([65, NQ], F32, tag="av0", bufs=1, name="av0")
            av1 = ps.tile([65, NQ], F32, tag="av1", bufs=1, name="av1")

            def den_pieces(hp):
                """Normalize pair hp's heads; interleaved into next pass.
                Reciprocal runs in place on the single den row; one DRAM
                bounce broadcasts 1/den across 64 partitions."""
                h0, h1 = 2 * hp, 2 * hp + 1
                dbs = [None, None]

                def recip(e):
                    def f():
                        h = 2 * hp + e
                        with nc.allow_low_precision(reason="bf16 softmax denominators"):
                            nc.vector.reciprocal(out=numT[h][64:65, :],
                                                 in_=numT[h][64:65, :])
                        nc.sync.dma_start(out=dsb_d[h, :],
                                          in_=numT[h][64:65, :])
                    return f

                def load_bc(e):
                    def f():
                        h = 2 * hp + e
                        den_bc = W.tile([64, NQ], BF16, tag="denb", bufs=2,
                                        name=f"denb{h}")
                        dbs[e] = den_bc
                        nc.sync.dma_start(
                            out=den_bc[:],
                            in_=bass.AP(tensor=dsb_d.ap().tensor, offset=h * NQ,
                                        ap=[[0, 64], [1, NQ]]))
                    return f

                def mul_half(e, half):
                    def f():
                        sl = slice(half * 512, (half + 1) * 512)
                        if e == 0:
                            nc.vector.tensor_mul(out=pairT[hp][0:64, sl],
                                                 in0=numT[h0][0:64, sl],
                                                 in1=dbs[0][:, sl])
                        else:
                            nc.vector.tensor_mul(out=numT[h1][0:64, sl],
                                                 in0=numT[h1][0:64, sl],
                                                 in1=dbs[1][:, sl])
                    return f

                def stitch():
                    nc.sync.dma_start(out=pairT[hp][64:128, :],
                                      in_=numT[h1][0:64, :])

                return [recip(0), recip(1), load_bc(0), load_bc(1),
                        mul_half(0, 0), mul_half(0, 1),
                        mul_half(1, 0), mul_half(1, 1), stitch]

            pend = []
            it_idx = 0
            for hp in range(4):
                h0, h1 = 2 * hp, 2 * hp + 1
                for jt in range(NT):
                    for ic in range(2):
                        i5 = ic * 512
                        sp = ps.tile([128, NQ], F32, tag="sp")
                        nc.tensor.matmul(
                            sp[:, 0:512],
                            KTp[hp][0:64, jt * 128:(jt + 1) * 128],
                            QTp[hp][0:64, i5:i5 + 512],
                            start=True, stop=True, tile_position=(0, 0))
                        nc.tensor.matmul(
                            sp[:, 512:1024],
                            KTp[hp][64:128, jt * 128:(jt + 1) * 128],
                            QTp[hp][64:128, i5:i5 + 512],
                            start=True, stop=True, tile_position=(64, 0))
                        eb = W.tile([128, NQ], BF16, tag="eb", bufs=4)
                        nc.scalar.activation(out=eb[:], in_=sp[:], func=AF.Exp)
                        pb = W.tile([128, NQ], BF16, tag="pb", bufs=4)
                        ea = expA[jt][:, i5:i5 + 512].unsqueeze(1) \
                            .broadcast_to([128, 2, 512])
                        nc.vector.tensor_tensor(
                            out=pb[:].rearrange("p (a b) -> p a b", a=2),
                            in0=eb[:].rearrange("p (a b) -> p a b", a=2),
                            in1=ea, op=OP.mult)
                        it_idx += 1
                        nc.tensor.matmul(av0[:, i5:i5 + 512],
                                         Vau[jt][:, h0, 0:65], pb[:, 0:512],
                                         start=(jt == 0), stop=(jt == NT - 1))
                        nc.tensor.matmul(av1[:, i5:i5 + 512],
                                         Vau[jt][:, h1, 0:65], pb[:, 512:1024],
                                         start=(jt == 0), stop=(jt == NT - 1))
                    if prod_by_hp[hp]:
                        prod_by_hp[hp].pop(0)()
                    while pend:
                        f = pend.pop(0)
                        if f is not None:
                            f()
                            break
                for f in pend:
                    if f is not None:
                        f()
                nc.vector.tensor_copy(out=numT[h0][:], in_=av0[:])
                nc.scalar.copy(out=numT[h1][:], in_=av1[:])
                pend = den_pieces(hp)
            for f in pend:
                if f is not None:
                    f()

        if dbg:
            dbg_xnT = nc.dram_tensor("dbg_xnT", [128, 4 * N], BF16,
                                     kind="ExternalOutput")
            nc.sync.dma_start(out=dbg_xnT[:, :],
                              in_=xnT[:].rearrange("p a b -> p (a b)"))
            dbg_QT = nc.dram_tensor("dbg_QT", [128, NQ], BF16,
                                    kind="ExternalOutput")
            nc.sync.dma_start(out=dbg_QT[:, :], in_=QTp[0][:])
            dbg_KT = nc.dram_tensor("dbg_KT", [128, N], BF16,
                                    kind="ExternalOutput")
            nc.sync.dma_start(out=dbg_KT[:, :], in_=KTp[0][:])
            dbg_V = nc.dram_tensor("dbg_V", [128, H * 66], BF16,
                                   kind="ExternalOutput")
            nc.sync.dma_start(out=dbg_V[:, :],
                              in_=Vau[0][:].rearrange("p a b -> p (a b)"))
            dbg_nT = nc.dram_tensor("dbg_nT", [65, NQ], BF16,
                                    kind="ExternalOutput")
            nc.sync.dma_start(out=dbg_nT[:, :], in_=numT[0][:])
            dbg_pT = nc.dram_tensor("dbg_pT", [128, NQ], BF16,
                                    kind="ExternalOutput")
            nc.sync.dma_start(out=dbg_pT[:, :], in_=pairT[0][:])

        # ---- output projection ---------------------------------------------
        with tc.tile_pool(name="psD", bufs=1, space="PSUM") as psD:
            for isl in range(8):
                yp = psD.tile([128, D], F32, tag="yp", bufs=4, name=f"yp{isl}")
                for hp in range(4):
                    nc.tensor.matmul(yp[:],
                                     pairT[hp][:, isl * 128:(isl + 1) * 128],
                                     woutP[hp][:],
                                     start=(hp == 0), stop=(hp == 3))
                ysb = W.tile([128, D], F32, tag="ysb", bufs=4)
                if has_b:
                    nc.vector.tensor_add(out=ysb[:], in0=yp[:], in1=bout_bc[:])
                elif isl % 2 == 0:
                    nc.vector.tensor_copy(out=ysb[:], in_=yp[:])
                else:
                    nc.scalar.copy(out=ysb[:], in_=yp[:])
                eng = nc.sync if isl % 2 == 0 else nc.scalar
                eng.dma_start(out=y_d[isl * 128:(isl + 1) * 128, :],
                              in_=ysb[:])
    if split:
        _split_waits(nc)
    return nc


_NC_CACHE = {}


def _get_nc(has_c, has_b):
    key = (has_c, has_b)
    if key not in _NC_CACHE:
        _NC_CACHE[key] = build(has_c, has_b)
    return _NC_CACHE[key]


LAST_EXEC_TIME_NS = None


def kernel(x, gamma, beta, Wqkv, Wout, bout, rel_table, temporal_mask,
           trace=True):
    global LAST_EXEC_TIME_NS
    x = np.asarray(x, np.float32)
    gamma = np.asarray(gamma, np.float32)
    beta = np.asarray(beta, np.float32)
    Wqkv = np.asarray(Wqkv, np.float32)
    Wout = np.asarray(Wout, np.float32)
    bout = np.asarray(bout, np.float32)
    rel_table = np.asarray(rel_table, np.float32)
    temporal_mask = np.asarray(temporal_mask)

    scale = DH ** -0.5
    w_eff = (Wqkv * gamma[:, None]).copy()
    w_eff[:, :D] *= scale
    cqkv = (beta @ Wqkv).astype(np.float32)
    cqkv[:D] *= scale
    wqkv_bf = w_eff.astype(ml_dtypes.bfloat16)
    wout_bf = Wout.astype(ml_dtypes.bfloat16)
    mask01T = (temporal_mask != 0).T  # [key j, query i]

    exp_table = np.exp(rel_table).astype(np.float32)
    keyperm_half = [
        np.concatenate([np.arange(i0, i0 + NQ),
                        np.arange(NQ - i0, NQ - i0 + NQ)])
        for i0 in (0, NQ)
    ]
    expa_half = []
    for half in range(2):
        kp = keyperm_half[half]
        d = (half * NQ + np.arange(NQ))[None, :] - kp[:, None]
        idx = np.clip(d, -(MAXREL - 1), MAXREL - 1) + MAXREL - 1
        ea = exp_table[idx] * mask01T[kp][:, half * NQ:(half + 1) * NQ]
        expa_half.append(ea.astype(ml_dtypes.bfloat16))

    in_maps = []
    for c in range(NCORES):
        b, half = c // 2, c % 2
        xp = np.ascontiguousarray(
            x[b][keyperm_half[half]]).astype(ml_dtypes.bfloat16)
        in_maps.append({
            "x": xp,
            "wqkv": wqkv_bf,
            "cqkv": cqkv,
            "wout": wout_bf,
            "bout": bout,
            "expa": expa_half[half],
        })

    nc = _get_nc(bool(np.any(cqkv != 0.0)), bool(np.any(bout != 0.0)))
    res = run_bass_kernel_spmd(nc, in_maps, core_ids=list(range(NCORES)),
                               trace=trace)
    LAST_EXEC_TIME_NS = res.exec_time_ns

    out = np.empty((B, N, D), np.float32)
    for c in range(NCORES):
        b, half = c // 2, c % 2
        out[b, half * NQ:(half + 1) * NQ] = res.results[c]["y"]
    return out


# revision 21
# speedup vs baseline: 1.3260x; 1.1115x over previous
"""Trainium2 Bass kernel for nn_Attention_8323646620215.

LayerNorm -> QKV -> scores(+rel-bias+mask) -> softmax -> attn@V -> out proj.

Sharding: 8 cores = (batch b in 0..3) x (query-half in 0..1). Each core
computes the full K/V for its batch and attention for its 1024 query rows;
no cross-core communication.

Fully software-pipelined single-pass schedule:
  - x arrives as bf16 (host-cast), LN on DVE, xn transposed via the DMA
    XBAR (dma_start_transpose) straight into xnT -- no PE transposes.
  - exp(rel-bias)*mask is precomputed on the host and DMA'd as bf16
    [N, NQ] tiles (expA); the mask enters multiplicatively:
    P = exp(S) * expA.
  - QKV production (PE matmuls + psum->SBUF copies) is interleaved into
    the attention head-pair passes so the PE never drains.
  - Per head-pair pass: S_T = K_T^T Q_T (two 64-row PE tiles), ACT exp,
    DVE/Pool multiply by expA, AV accumulate with an augmented ones-row
    for the softmax denominators.
  - Denominators: broadcast-DMA the den row across 64 partitions, then a
    single tensor_tensor divide per head (no DRAM round trip).
  - Output projection immediately follows the last pass.

LN gamma / attention scale / beta are folded into the QKV weights
host-side. The host permutes each core's tokens so its own query half
comes first, keeping the device program identical across cores.
"""
import sys
import types
import numpy as np

sys.path.insert(0, "/opt/trn_rl_repo")

# ---- environment fixes (axon agent container) -------------------------------
if "antenv.axon_hooks" not in sys.modules:
    _m = types.ModuleType("antenv.axon_hooks")
    _m._hook = None
    _m.set_axon_ntff_profile_hook = lambda h: setattr(_m, "_hook", h)
    _m.get_axon_ntff_profile_hook = lambda: _m._hook
    sys.modules["antenv.axon_hooks"] = _m
    try:
        from trn_agent_boot.trn_boot import _ntff_profile_via_ctypes
        _m._hook = _ntff_profile_via_ctypes("/opt/axon/libaxon_pjrt.so")
    except Exception:
        pass

import ml_dtypes  # noqa: E402
from concourse import bass, mybir, tile  # noqa: E402
from concourse.bass_utils import run_bass_kernel_spmd  # noqa: E402

F32 = mybir.dt.float32
BF16 = mybir.dt.bfloat16
AF = mybir.ActivationFunctionType
OP = mybir.AluOpType

B, N, D, H, DH, MAXREL = 4, 2048, 512, 8, 64, 200
NQ = N // 2          # queries per core
NT = N // 128        # 16 token tiles
NCORES = 8

# This container's walrus rejects instructions with more than one sem wait.
# Splitting is sound: a same-engine NoOp right before the instruction
# enforces the wait at the same program point (sequencers run in order).


def _split_waits(nc, maxw=1):
    n_split = 0
    for f in nc.m.functions:
        for blk in f.blocks:
            bb = blk.bb if hasattr(blk, "bb") else blk
            insts = list(bb.instructions)
            out = []
            changed = False
            for inst in insts:
                si = inst.sync_info
                waits = list(si.on_wait) if si and si.on_wait else []
                if len(waits) > maxw:
                    extra = waits[:-maxw]
                    chunks = [extra[j:j + maxw] for j in range(0, len(extra), maxw)]
                    for i, chunk in enumerate(chunks):
                        nop = mybir.InstNoOp(name=f"{inst.name}-ws{i}", ins=[], outs=[])
                        nop.engine = inst.engine
                        nop.sync_info = mybir.SyncInfo(on_wait=chunk, on_update=[])
                        out.append(nop)
                    si.on_wait = waits[-maxw:]
                    changed = True
                    n_split += 1
                out.append(inst)
            if changed:
                bb.instructions = out
    return n_split


def build(has_c=False, has_b=False, split=True, dbg=False):
    nc = bass.Bass("TRN2", target_bir_lowering=False, debug=False,
                   num_devices=NCORES)
    x_d = nc.dram_tensor("x", [N, D], BF16, kind="ExternalInput")
    wqkv_d = nc.dram_tensor("wqkv", [D, 3 * D], BF16, kind="ExternalInput")
    cqkv_d = nc.dram_tensor("cqkv", [3 * D], F32, kind="ExternalInput")
    wout_d = nc.dram_tensor("wout", [D, D], BF16, kind="ExternalInput")
    bout_d = nc.dram_tensor("bout", [D], F32, kind="ExternalInput")
    expa_d = nc.dram_tensor("expa", [N, NQ], BF16, kind="ExternalInput")
    dsb_d = nc.dram_tensor("den_scratch", [H, NQ], BF16)
    dsi_d = nc.dram_tensor("invden_scratch", [H, NQ], BF16)
    y_d = nc.dram_tensor("y", [NQ, D], F32, kind="ExternalOutput")

    with tile.TileContext(nc) as tc, \
         tc.tile_pool(name="const", bufs=1) as C, \
         tc.tile_pool(name="pers", bufs=1) as P, \
         tc.tile_pool(name="work", bufs=3) as W:

        # ---- persistent tiles ----------------------------------------------
        xnT = P.tile([128, 4, N], BF16, tag="xnT", name="xnT")
        KTp = [P.tile([128, N], BF16, tag=f"KT{hp}", name=f"KT{hp}") for hp in range(4)]
        QTp = [P.tile([128, NQ], BF16, tag=f"QT{hp}", name=f"QT{hp}") for hp in range(4)]
        Vau = [P.tile([128, H, 66], BF16, tag=f"V{t}", name=f"Vau{t}") for t in range(NT)]
        expA = [P.tile([128, NQ], BF16, tag=f"eA{t}", name=f"eA{t}") for t in range(NT)]
        numT = [P.tile([65, NQ], BF16, tag=f"nT{h}", name=f"nT{h}") for h in range(H)]
        pairT = [P.tile([128, NQ], BF16, tag=f"pT{hp}", name=f"pT{hp}") for hp in range(4)]

        wqkv_sb = [C.tile([128, 3 * D], BF16, tag=f"wqkv{kb}", name=f"wqkv{kb}") for kb in range(4)]
        woutP = [C.tile([128, D], BF16, tag=f"woutP{hp}", name=f"woutP{hp}") for hp in range(4)]

        # ---- input DMAs -----------------------------------------------------
        # sync (SP HWDGE): x tiles then expA tiles (+ later den broadcasts, y).
        # scalar (ACT HWDGE): XBAR transposes of xn (issued inline in LN flow).
        # gpsimd (SWDGE): weights + small constants.
        x_ts = [W.tile([128, D], BF16, tag="x", bufs=6, name=f"x{t}") for t in range(NT)]
        for t in range(8):
            nc.sync.dma_start(out=x_ts[t][:], in_=x_d[t * 128:(t + 1) * 128, :])
        for kb in range(4):
            nc.gpsimd.dma_start(out=wqkv_sb[kb][:],
                                in_=wqkv_d[kb * 128:(kb + 1) * 128, :])
        for t in range(8, NT):
            nc.gpsimd.dma_start(out=x_ts[t][:],
                                in_=x_d[t * 128:(t + 1) * 128, :])
        for t in range(NT):
            nc.gpsimd.dma_start(out=expA[t][:],
                                in_=expa_d[t * 128:(t + 1) * 128, :])
        for hp in range(4):
            nc.gpsimd.dma_start(out=woutP[hp][:],
                                in_=wout_d[hp * 128:(hp + 1) * 128, :])
        cq_all = C.tile([128, 12], F32, tag="cq")
        cv_bc = C.tile([128, D], F32, tag="cv")
        bout_bc = C.tile([128, D], F32, tag="bout")
        if has_c:
            nc.gpsimd.dma_start(
                out=cq_all[:],
                in_=bass.AP(tensor=cqkv_d.ap().tensor, offset=0,
                            ap=[[1, 128], [128, 12]]))
            nc.gpsimd.dma_start(
                out=cv_bc[:],
                in_=bass.AP(tensor=cqkv_d.ap().tensor, offset=2 * D,
                            ap=[[0, 128], [1, D]]))
        if has_b:
            nc.gpsimd.dma_start(
                out=bout_bc[:],
                in_=bass.AP(tensor=bout_d.ap().tensor, offset=0,
                            ap=[[0, 128], [1, D]]))

        eps_t = C.tile([128, 1], F32, tag="eps")
        nc.vector.memset(eps_t[:], 1e-5)
        for t in range(NT):
            nc.gpsimd.memset(Vau[t][:, :, 64:66], 1.0)

        with tc.tile_pool(name="ps", bufs=2, space="PSUM") as ps:
            # ---- LayerNorm + XBAR transpose (prologue, DVE+ACT) ------------
            def ln_tile(t):
                st = W.tile([128, 6], F32, tag="st")
                nc.vector.bn_stats(out=st[:], in_=x_ts[t][:])
                mv = W.tile([128, 2], F32, tag="mv", bufs=4)
                nc.vector.bn_aggr(out=mv[:], in_=st[:])
                rs = W.tile([128, 1], F32, tag="rs", bufs=4)
                nc.scalar.activation(out=rs[:], in_=mv[:, 1:2], func=AF.Sqrt,
                                     bias=eps_t[:])
                nc.vector.reciprocal(out=rs[:], in_=rs[:])
                xn = W.tile([128, D], BF16, tag="xn", bufs=3)
                nc.vector.tensor_scalar(out=xn[:], in0=x_ts[t][:],
                                        scalar1=mv[:, 0:1],
                                        scalar2=rs[:],
                                        op0=OP.subtract, op1=OP.mult)
                nc.scalar.dma_start_transpose(
                    xnT[:, :, t * 128:(t + 1) * 128], xn[:])

            # ---- QKV production tasks (PE + copy), interleaved -------------
            def q_task(hp):
                def f():
                    qp = ps.tile([128, NQ], F32, tag="sp")
                    for ic in range(2):
                        for kb in range(4):
                            nc.tensor.matmul(
                                qp[:, ic * 512:(ic + 1) * 512],
                                wqkv_sb[kb][:, hp * 128:(hp + 1) * 128],
                                xnT[:, kb, ic * 512:(ic + 1) * 512],
                                start=(kb == 0), stop=(kb == 3))
                    if has_c:
                        nc.vector.tensor_scalar_add(
                            out=QTp[hp][:], in0=qp[:],
                            scalar1=cq_all[:, hp:hp + 1])
                    else:
                        nc.scalar.copy(out=QTp[hp][:], in_=qp[:])
                return f

            def k_task(hp, half):
                def f():
                    kp = ps.tile([128, NQ], F32, tag="sp")
                    for tc4 in range(2):
                        for kb in range(4):
                            nc.tensor.matmul(
                                kp[:, tc4 * 512:(tc4 + 1) * 512],
                                wqkv_sb[kb][:, D + hp * 128:D + (hp + 1) * 128],
                                xnT[:, kb,
                                    half * 1024 + tc4 * 512:
                                    half * 1024 + (tc4 + 1) * 512],
                                start=(kb == 0), stop=(kb == 3))
                    if has_c:
                        nc.vector.tensor_scalar_add(
                            out=KTp[hp][:, half * 1024:(half + 1) * 1024],
                            in0=kp[:], scalar1=cq_all[:, 4 + hp:5 + hp])
                    else:
                        nc.scalar.copy(
                            out=KTp[hp][:, half * 1024:(half + 1) * 1024],
                            in_=kp[:])
                return f

            def v_task(j2):
                def f():
                    vp = ps.tile([128, NQ], F32, tag="sp")
                    for e in range(2):
                        jt = 2 * j2 + e
                        for kb in range(4):
                            nc.tensor.matmul(
                                vp[:, e * 512:(e + 1) * 512],
                                xnT[:, kb, jt * 128:(jt + 1) * 128],
                                wqkv_sb[kb][:, 2 * D:3 * D],
                                start=(kb == 0), stop=(kb == 3))
                    for e in range(2):
                        jt = 2 * j2 + e
                        src = vp[:, e * 512:(e + 1) * 512].rearrange(
                            "p (h d) -> p h d", h=H)
                        if has_c:
                            nc.vector.tensor_add(
                                out=Vau[jt][:, :, 0:64], in0=src,
                                in1=cv_bc[:].rearrange("p (h d) -> p h d", h=H))
                        else:
                            nc.scalar.copy(out=Vau[jt][:, :, 0:64], in_=src)
                return f

            # Emit LN for the query half, then enough QKV to start attention.
            for t in range(4):
                ln_tile(t)
            for t in range(4, 8):
                ln_tile(t)
            q_task(0)()
            k_task(0, 0)()
            for t in range(8, 12):
                ln_tile(t)
            v_task(0)()
            v_task(1)()
            for t in range(12, 16):
                ln_tile(t)

            # prod tasks popped one per jt during passes; V(j2) must land
            # before jt=2*j2 of hp0, K/Q for pass hp before that pass starts.
            prod_by_hp = {
                0: [v_task(2), v_task(3), v_task(4), v_task(5), k_task(0, 1),
                    v_task(6), v_task(7), q_task(1), k_task(1, 0),
                    k_task(1, 1)],
                1: [q_task(2), k_task(2, 0), k_task(2, 1)],
                2: [q_task(3), k_task(3, 0), k_task(3, 1)],
                3: [],
            }

            # ---- attention passes ------------------------------------------
            av0 = ps.tile([65, NQ], F32, tag="av0", bufs=1, name="av0")
            av1 = ps.tile([65, NQ], F32, tag="av1", bufs=1, name="av1")

            def den_pieces(hp):
                """Normalize pair hp's heads; interleaved into next pass.
                Reciprocal runs in place on the single den row; one DRAM
                bounce broadcasts 1/den across 64 partitions."""
                h0, h1 = 2 * hp, 2 * hp + 1
                dbs = [None, None]

                def recip(e):
                    def f():
                        h = 2 * hp + e
                        with nc.allow_low_precision(reason="bf16 softmax denominators"):
                            nc.vector.reciprocal(out=numT[h][64:65, :],
                                                 in_=numT[h][64:65, :])
                        nc.sync.dma_start(out=dsb_d[h, :],
                                          in_=numT[h][64:65, :])
                    return f

                def load_bc(e):
                    def f():
                        h = 2 * hp + e
                        den_bc = W.tile([64, NQ], BF16, tag="denb", bufs=2,
                                        name=f"denb{h}")
                        dbs[e] = den_bc
                        nc.sync.dma_start(
                            out=den_bc[:],
                            in_=bass.AP(tensor=dsb_d.ap().tensor, offset=h * NQ,
                                        ap=[[0, 64], [1, NQ]]))
                    return f

                def mul_half(e, half):
                    def f():
                        sl = slice(half * 512, (half + 1) * 512)
                        if e == 0:
                            nc.vector.tensor_mul(out=pairT[hp][0:64, sl],
                                                 in0=numT[h0][0:64, sl],
                                                 in1=dbs[0][:, sl])
                        else:
                            nc.vector.tensor_mul(out=numT[h1][0:64, sl],
                                                 in0=numT[h1][0:64, sl],
                                                 in1=dbs[1][:, sl])
                    return f

                def stitch():
                    nc.sync.dma_start(out=pairT[hp][64:128, :],
                                      in_=numT[h1][0:64, :])

                return [recip(0), recip(1), load_bc(0), load_bc(1),
                        mul_half(0, 0), mul_half(0, 1),
                        mul_half(1, 0), mul_half(1, 1), stitch]

            pend = []
            it_idx = 0
            for hp in range(4):
                h0, h1 = 2 * hp, 2 * hp + 1
                for jt in range(NT):
                    for ic in range(2):
                        i5 = ic * 512
                        sp = ps.tile([128, NQ], F32, tag="sp")
                        nc.tensor.matmul(
                            sp[:, 0:512],
                            KTp[hp][0:64, jt * 128:(jt + 1) * 128],
                            QTp[hp][0:64, i5:i5 + 512],
                            start=True, stop=True, tile_position=(0, 0))
                        nc.tensor.matmul(
                            sp[:, 512:1024],
                            KTp[hp][64:128, jt * 128:(jt + 1) * 128],
                            QTp[hp][64:128, i5:i5 + 512],
                            start=True, stop=True, tile_position=(64, 0))
                        eb = W.tile([128, NQ], BF16, tag="eb", bufs=4)
                        nc.scalar.activation(out=eb[:], in_=sp[:], func=AF.Exp)
                        pb = W.tile([128, NQ], BF16, tag="pb", bufs=4)
                        ea = expA[jt][:, i5:i5 + 512].unsqueeze(1) \
                            .broadcast_to([128, 2, 512])
                        nc.vector.tensor_tensor(
                            out=pb[:].rearrange("p (a b) -> p a b", a=2),
                            in0=eb[:].rearrange("p (a b) -> p a b", a=2),
                            in1=ea, op=OP.mult)
                        it_idx += 1
                        nc.tensor.matmul(av0[:, i5:i5 + 512],
                                         Vau[jt][:, h0, 0:65], pb[:, 0:512],
                                         start=(jt == 0), stop=(jt == NT - 1))
                        nc.tensor.matmul(av1[:, i5:i5 + 512],
                                         Vau[jt][:, h1, 0:65], pb[:, 512:1024],
                                         start=(jt == 0), stop=(jt == NT - 1))
                    if prod_by_hp[hp]:
                        prod_by_hp[hp].pop(0)()
                    while pend:
                        f = pend.pop(0)
                        if f is not None:
                            f()
                            break
                for f in pend:
                    if f is not None:
                        f()
                nc.vector.tensor_copy(out=numT[h0][:], in_=av0[:])
                nc.scalar.copy(out=numT[h1][:], in_=av1[:])
                pend = den_pieces(hp)
            for f in pend:
                if f is not None:
                    f()

        if dbg:
            dbg_xnT = nc.dram_tensor("dbg_xnT", [128, 4 * N], BF16,
                                     kind="ExternalOutput")
            nc.sync.dma_start(out=dbg_xnT[:, :],
                              in_=xnT[:].rearrange("p a b -> p (a b)"))
            dbg_QT = nc.dram_tensor("dbg_QT", [128, NQ], BF16,
                                    kind="ExternalOutput")
            nc.sync.dma_start(out=dbg_QT[:, :], in_=QTp[0][:])
            dbg_KT = nc.dram_tensor("dbg_KT", [128, N], BF16,
                                    kind="ExternalOutput")
            nc.sync.dma_start(out=dbg_KT[:, :], in_=KTp[0][:])
            dbg_V = nc.dram_tensor("dbg_V", [128, H * 66], BF16,
                                   kind="ExternalOutput")
            nc.sync.dma_start(out=dbg_V[:, :],
                              in_=Vau[0][:].rearrange("p a b -> p (a b)"))
            dbg_nT = nc.dram_tensor("dbg_nT", [65, NQ], BF16,
                                    kind="ExternalOutput")
            nc.sync.dma_start(out=dbg_nT[:, :], in_=numT[0][:])
            dbg_pT = nc.dram_tensor("dbg_pT", [128, NQ], BF16,
                                    kind="ExternalOutput")
            nc.sync.dma_start(out=dbg_pT[:, :], in_=pairT[0][:])

        # ---- output projection ---------------------------------------------
        with tc.tile_pool(name="psD", bufs=1, space="PSUM") as psD:
            for isl in range(8):
                yp = psD.tile([128, D], F32, tag="yp", bufs=4, name=f"yp{isl}")
                for hp in range(4):
                    nc.tensor.matmul(yp[:],
                                     pairT[hp][:, isl * 128:(isl + 1) * 128],
                                     woutP[hp][:],
                                     start=(hp == 0), stop=(hp == 3))
                ysb = W.tile([128, D], F32, tag="ysb", bufs=4)
                if has_b:
                    nc.vector.tensor_add(out=ysb[:], in0=yp[:], in1=bout_bc[:])
                elif isl % 2 == 0:
                    nc.vector.tensor_copy(out=ysb[:], in_=yp[:])
                else:
                    nc.scalar.copy(out=ysb[:], in_=yp[:])
                eng = nc.sync if isl % 2 == 0 else nc.scalar
                eng.dma_start(out=y_d[isl * 128:(isl + 1) * 128, :],
                              in_=ysb[:])
    if split:
        _split_waits(nc)
    return nc


_NC_CACHE = {}


def _get_nc(has_c, has_b):
    key = (has_c, has_b)
    if key not in _NC_CACHE:
        _NC_CACHE[key] = build(has_c, has_b)
    return _NC_CACHE[key]


LAST_EXEC_TIME_NS = None


def kernel(x, gamma, beta, Wqkv, Wout, bout, rel_table, temporal_mask,
           trace=True):
    global LAST_EXEC_TIME_NS
    x = np.asarray(x, np.float32)
    gamma = np.asarray(gamma, np.float32)
    beta = np.asarray(beta, np.float32)
    Wqkv = np.asarray(Wqkv, np.float32)
    Wout = np.asarray(Wout, np.float32)
    bout = np.asarray(bout, np.float32)
    rel_table = np.asarray(rel_table, np.float32)
    temporal_mask = np.asarray(temporal_mask)

    scale = DH ** -0.5
    w_eff = (Wqkv * gamma[:, None]).copy()
    w_eff[:, :D] *= scale
    cqkv = (beta @ Wqkv).astype(np.float32)
    cqkv[:D] *= scale
    wqkv_bf = w_eff.astype(ml_dtypes.bfloat16)
    wout_bf = Wout.astype(ml_dtypes.bfloat16)
    mask01T = (temporal_mask != 0).T  # [key j, query i]

    exp_table = np.exp(rel_table).astype(np.float32)
    keyperm_half = [
        np.concatenate([np.arange(i0, i0 + NQ),
                        np.arange(NQ - i0, NQ - i0 + NQ)])
        for i0 in (0, NQ)
    ]
    expa_half = []
    for half in range(2):
        kp = keyperm_half[half]
        d = (half * NQ + np.arange(NQ))[None, :] - kp[:, None]
        idx = np.clip(d, -(MAXREL - 1), MAXREL - 1) + MAXREL - 1
        ea = exp_table[idx] * mask01T[kp][:, half * NQ:(half + 1) * NQ]
        expa_half.append(ea.astype(ml_dtypes.bfloat16))

    in_maps = []
    for c in range(NCORES):
        b, half = c // 2, c % 2
        xp = np.ascontiguousarray(
            x[b][keyperm_half[half]]).astype(ml_dtypes.bfloat16)
        in_maps.append({
            "x": xp,
            "wqkv": wqkv_bf,
            "cqkv": cqkv,
            "wout": wout_bf,
            "bout": bout,
            "expa": expa_half[half],
        })

    nc = _get_nc(bool(np.any(cqkv != 0.0)), bool(np.any(bout != 0.0)))
    res = run_bass_kernel_spmd(nc, in_maps, core_ids=list(range(NCORES)),
                               trace=trace)
    LAST_EXEC_TIME_NS = res.exec_time_ns

    out = np.empty((B, N, D), np.float32)
    for c in range(NCORES):
        b, half = c // 2, c % 2
        out[b, half * NQ:(half + 1) * NQ] = res.results[c]["y"]
    return out


# revision 22
# speedup vs baseline: 1.5489x; 1.1682x over previous
"""Trainium2 Bass kernel for nn_Attention_8323646620215.

LayerNorm -> QKV -> scores(+rel-bias+mask) -> softmax -> attn@V -> out proj.

Sharding: 8 cores = (batch b in 0..3) x (query-half in 0..1). Each core
computes the full K/V for its batch and attention for its 1024 query rows;
no cross-core communication.

Fully software-pipelined single-pass schedule:
  - x arrives as bf16 (host-cast), LN on DVE, xn transposed via the DMA
    XBAR (dma_start_transpose) straight into xnT -- no PE transposes.
  - exp(rel-bias)*mask is precomputed on the host and DMA'd as bf16
    [N, NQ] tiles (expA); the mask enters multiplicatively:
    P = exp(S) * expA.
  - QKV production (PE matmuls + psum->SBUF copies) is interleaved into
    the attention head-pair passes so the PE never drains.
  - Per head-pair pass: S_T = K_T^T Q_T (two 64-row PE tiles), ACT exp,
    DVE/Pool multiply by expA, AV accumulate with an augmented ones-row
    for the softmax denominators.
  - Denominators: broadcast-DMA the den row across 64 partitions, then a
    single tensor_tensor divide per head (no DRAM round trip).
  - Output projection immediately follows the last pass.

LN gamma / attention scale / beta are folded into the QKV weights
host-side. The host permutes each core's tokens so its own query half
comes first, keeping the device program identical across cores.
"""
import sys
import types
import numpy as np

sys.path.insert(0, "/opt/trn_rl_repo")

# ---- environment fixes (axon agent container) -------------------------------
if "antenv.axon_hooks" not in sys.modules:
    _m = types.ModuleType("antenv.axon_hooks")
    _m._hook = None
    _m.set_axon_ntff_profile_hook = lambda h: setattr(_m, "_hook", h)
    _m.get_axon_ntff_profile_hook = lambda: _m._hook
    sys.modules["antenv.axon_hooks"] = _m
    try:
        from trn_agent_boot.trn_boot import _ntff_profile_via_ctypes
        _m._hook = _ntff_profile_via_ctypes("/opt/axon/libaxon_pjrt.so")
    except Exception:
        pass

import ml_dtypes  # noqa: E402
from concourse import bass, mybir, tile  # noqa: E402
from concourse.bass_utils import run_bass_kernel_spmd  # noqa: E402

F32 = mybir.dt.float32
BF16 = mybir.dt.bfloat16
AF = mybir.ActivationFunctionType
OP = mybir.AluOpType

B, N, D, H, DH, MAXREL = 4, 2048, 512, 8, 64, 200
NQ = N // 2          # queries per core
NT = N // 128        # 16 token tiles
NCORES = 8

# This container's walrus rejects instructions with more than one sem wait.
# Splitting is sound: a same-engine NoOp right before the instruction
# enforces the wait at the same program point (sequencers run in order).


def _split_waits(nc, maxw=1):
    n_split = 0
    for f in nc.m.functions:
        for blk in f.blocks:
            bb = blk.bb if hasattr(blk, "bb") else blk
            insts = list(bb.instructions)
            out = []
            changed = False
            for inst in insts:
                si = inst.sync_info
                waits = list(si.on_wait) if si and si.on_wait else []
                if len(waits) > maxw:
                    extra = waits[:-maxw]
                    chunks = [extra[j:j + maxw] for j in range(0, len(extra), maxw)]
                    for i, chunk in enumerate(chunks):
                        nop = mybir.InstNoOp(name=f"{inst.name}-ws{i}", ins=[], outs=[])
                        nop.engine = inst.engine
                        nop.sync_info = mybir.SyncInfo(on_wait=chunk, on_update=[])
                        out.append(nop)
                    si.on_wait = waits[-maxw:]
                    changed = True
                    n_split += 1
                out.append(inst)
            if changed:
                bb.instructions = out
    return n_split


def build(has_c=False, has_b=False, split=True, dbg=False):
    nc = bass.Bass("TRN2", target_bir_lowering=False, debug=False,
                   num_devices=NCORES)
    x_d = nc.dram_tensor("x", [N, D], BF16, kind="ExternalInput")
    wqkv_d = nc.dram_tensor("wqkv", [D, 3 * D], BF16, kind="ExternalInput")
    cqkv_d = nc.dram_tensor("cqkv", [3 * D], F32, kind="ExternalInput")
    wout_d = nc.dram_tensor("wout", [D, D], BF16, kind="ExternalInput")
    bout_d = nc.dram_tensor("bout", [D], F32, kind="ExternalInput")
    expa_d = nc.dram_tensor("expa", [N, NQ], BF16, kind="ExternalInput")
    dsb_d = nc.dram_tensor("den_scratch", [H, NQ], BF16)
    dsi_d = nc.dram_tensor("invden_scratch", [H, NQ], BF16)
    y_d = nc.dram_tensor("y", [NQ, D], F32, kind="ExternalOutput")

    with tile.TileContext(nc) as tc, \
         tc.tile_pool(name="const", bufs=1) as C, \
         tc.tile_pool(name="pers", bufs=1) as P, \
         tc.tile_pool(name="work", bufs=3) as W:

        # ---- persistent tiles ----------------------------------------------
        xnT = P.tile([128, 4, N], BF16, tag="xnT", name="xnT")
        KTp = [P.tile([128, N], BF16, tag=f"KT{hp}", name=f"KT{hp}") for hp in range(4)]
        QTp = [P.tile([128, NQ], BF16, tag=f"QT{hp}", name=f"QT{hp}") for hp in range(4)]
        Vau = [P.tile([128, H, 66], BF16, tag=f"V{t}", name=f"Vau{t}") for t in range(NT)]
        expA = [P.tile([128, NQ], BF16, tag=f"eA{t}", name=f"eA{t}") for t in range(NT)]
        numT = [P.tile([65, NQ], BF16, tag=f"nT{h}", name=f"nT{h}") for h in range(H)]
        pairT = [P.tile([128, NQ], BF16, tag=f"pT{hp}", name=f"pT{hp}") for hp in range(4)]

        wqkv_sb = [C.tile([128, 3 * D], BF16, tag=f"wqkv{kb}", name=f"wqkv{kb}") for kb in range(4)]
        woutP = [C.tile([128, D], BF16, tag=f"woutP{hp}", name=f"woutP{hp}") for hp in range(4)]

        # ---- input DMAs -----------------------------------------------------
        # sync (SP HWDGE): x tiles then expA tiles (+ later den broadcasts, y).
        # scalar (ACT HWDGE): XBAR transposes of xn (issued inline in LN flow).
        # gpsimd (SWDGE): weights + small constants.
        x_ts = [W.tile([128, D], BF16, tag="x", bufs=8, name=f"x{t}") for t in range(NT)]
        for t in range(8):
            nc.sync.dma_start(out=x_ts[t][:], in_=x_d[t * 128:(t + 1) * 128, :])
        for kb in range(4):
            nc.gpsimd.dma_start(out=wqkv_sb[kb][:],
                                in_=wqkv_d[kb * 128:(kb + 1) * 128, :])
        for t in range(8, NT):
            nc.gpsimd.dma_start(out=x_ts[t][:],
                                in_=x_d[t * 128:(t + 1) * 128, :])
        for t in range(NT):
            nc.gpsimd.dma_start(out=expA[t][:],
                                in_=expa_d[t * 128:(t + 1) * 128, :])
        for hp in range(4):
            nc.gpsimd.dma_start(out=woutP[hp][:],
                                in_=wout_d[hp * 128:(hp + 1) * 128, :])
        cq_all = C.tile([128, 12], F32, tag="cq")
        cv_bc = C.tile([128, D], F32, tag="cv")
        bout_bc = C.tile([128, D], F32, tag="bout")
        if has_c:
            nc.gpsimd.dma_start(
                out=cq_all[:],
                in_=bass.AP(tensor=cqkv_d.ap().tensor, offset=0,
                            ap=[[1, 128], [128, 12]]))
            nc.gpsimd.dma_start(
                out=cv_bc[:],
                in_=bass.AP(tensor=cqkv_d.ap().tensor, offset=2 * D,
                            ap=[[0, 128], [1, D]]))
        if has_b:
            nc.gpsimd.dma_start(
                out=bout_bc[:],
                in_=bass.AP(tensor=bout_d.ap().tensor, offset=0,
                            ap=[[0, 128], [1, D]]))

        eps_t = C.tile([128, 1], F32, tag="eps")
        nc.vector.memset(eps_t[:], 1e-5)
        for t in range(NT):
            nc.gpsimd.memset(Vau[t][:, :, 64:66], 1.0)

        with tc.tile_pool(name="ps", bufs=2, space="PSUM") as ps:
            # ---- LayerNorm + XBAR transpose (prologue) ---------------------
            # Staged per group of 4 tiles: stats+sqrt first (DVE+ACT,
            # independent), then apply+transpose (DVE + ACT-queue xbar) so
            # the xbar issues never gate the next tile's sqrt.
            mvs = [None] * NT
            rss = [None] * NT

            def ln_stats(t):
                st = W.tile([128, 6], F32, tag="st")
                nc.vector.bn_stats(out=st[:], in_=x_ts[t][:])
                mv = W.tile([128, 2], F32, tag="mv", bufs=8)
                nc.vector.bn_aggr(out=mv[:], in_=st[:])
                rs = W.tile([128, 1], F32, tag="rs", bufs=8)
                nc.scalar.activation(out=rs[:], in_=mv[:, 1:2], func=AF.Sqrt,
                                     bias=eps_t[:])
                mvs[t], rss[t] = mv, rs

            def ln_apply(t):
                nc.vector.reciprocal(out=rss[t][:], in_=rss[t][:])
                xn = W.tile([128, D], BF16, tag="xn", bufs=3)
                nc.vector.tensor_scalar(out=xn[:], in0=x_ts[t][:],
                                        scalar1=mvs[t][:, 0:1],
                                        scalar2=rss[t][:],
                                        op0=OP.subtract, op1=OP.mult)
                nc.scalar.dma_start_transpose(
                    xnT[:, :, t * 128:(t + 1) * 128], xn[:])

            def ln_tile(t):
                ln_stats(t)
                ln_apply(t)

            # ---- QKV production tasks (PE + copy), interleaved -------------
            def q_task(hp):
                def f():
                    qp = ps.tile([128, NQ], F32, tag="sp")
                    for ic in range(2):
                        for kb in range(4):
                            nc.tensor.matmul(
                                qp[:, ic * 512:(ic + 1) * 512],
                                wqkv_sb[kb][:, hp * 128:(hp + 1) * 128],
                                xnT[:, kb, ic * 512:(ic + 1) * 512],
                                start=(kb == 0), stop=(kb == 3))
                    if has_c:
                        nc.vector.tensor_scalar_add(
                            out=QTp[hp][:], in0=qp[:],
                            scalar1=cq_all[:, hp:hp + 1])
                    else:
                        nc.scalar.copy(out=QTp[hp][:], in_=qp[:])
                return f

            def k_task(hp, half):
                def f():
                    kp = ps.tile([128, NQ], F32, tag="sp")
                    for tc4 in range(2):
                        for kb in range(4):
                            nc.tensor.matmul(
                                kp[:, tc4 * 512:(tc4 + 1) * 512],
                                wqkv_sb[kb][:, D + hp * 128:D + (hp + 1) * 128],
                                xnT[:, kb,
                                    half * 1024 + tc4 * 512:
                                    half * 1024 + (tc4 + 1) * 512],
                                start=(kb == 0), stop=(kb == 3))
                    if has_c:
                        nc.vector.tensor_scalar_add(
                            out=KTp[hp][:, half * 1024:(half + 1) * 1024],
                            in0=kp[:], scalar1=cq_all[:, 4 + hp:5 + hp])
                    else:
                        nc.scalar.copy(
                            out=KTp[hp][:, half * 1024:(half + 1) * 1024],
                            in_=kp[:])
                return f

            def v_task(j2):
                def f():
                    vp = ps.tile([128, NQ], F32, tag="sp")
                    for e in range(2):
                        jt = 2 * j2 + e
                        for kb in range(4):
                            nc.tensor.matmul(
                                vp[:, e * 512:(e + 1) * 512],
                                xnT[:, kb, jt * 128:(jt + 1) * 128],
                                wqkv_sb[kb][:, 2 * D:3 * D],
                                start=(kb == 0), stop=(kb == 3))
                    for e in range(2):
                        jt = 2 * j2 + e
                        src = vp[:, e * 512:(e + 1) * 512].rearrange(
                            "p (h d) -> p h d", h=H)
                        if has_c:
                            nc.vector.tensor_add(
                                out=Vau[jt][:, :, 0:64], in0=src,
                                in1=cv_bc[:].rearrange("p (h d) -> p h d", h=H))
                        else:
                            nc.scalar.copy(out=Vau[jt][:, :, 0:64], in_=src)
                return f

            # Emit LN for the query half, then enough QKV to start attention.
            for t in range(4):
                ln_stats(t)
            for t in range(4):
                ln_apply(t)
            for t in range(4, 8):
                ln_stats(t)
            for t in range(4, 8):
                ln_apply(t)
            q_task(0)()
            k_task(0, 0)()
            for t in range(8, 12):
                ln_stats(t)
            for t in range(8, 12):
                ln_apply(t)
            v_task(0)()
            v_task(1)()
            for t in range(12, 16):
                ln_stats(t)
            for t in range(12, 16):
                ln_apply(t)

            # prod tasks popped one per jt during passes; V(j2) must land
            # before jt=2*j2 of hp0, K/Q for pass hp before that pass starts.
            prod_by_hp = {
                0: [v_task(2), v_task(3), v_task(4), v_task(5), k_task(0, 1),
                    v_task(6), v_task(7), q_task(1), k_task(1, 0),
                    k_task(1, 1)],
                1: [q_task(2), k_task(2, 0), k_task(2, 1)],
                2: [q_task(3), k_task(3, 0), k_task(3, 1)],
                3: [],
            }

            # ---- attention passes ------------------------------------------
            av0 = ps.tile([65, NQ], F32, tag="av0", bufs=1, name="av0")
            av1 = ps.tile([65, NQ], F32, tag="av1", bufs=1, name="av1")

            def den_pieces(hp):
                """Normalize pair hp's heads; interleaved into next pass.
                Denominator rows bounce via DRAM into a [128,16] tile so the
                reciprocal runs wide, then broadcast across 64 partitions."""
                h0, h1 = 2 * hp, 2 * hp + 1
                dal = W.tile([128, 2 * NQ // 128], BF16, tag="dall", bufs=2,
                             name=f"dal{hp}")
                dbs = [None, None]

                def save_den(e):
                    def f():
                        h = 2 * hp + e
                        nc.sync.dma_start(out=dsb_d[h, :],
                                          in_=numT[h][64:65, :])
                    return f

                def p0():
                    nc.sync.dma_start(
                        out=dal[:],
                        in_=bass.AP(tensor=dsb_d.ap().tensor, offset=2 * hp * NQ,
                                    ap=[[2 * NQ // 128, 128], [1, 2 * NQ // 128]]))

                def p1():
                    with nc.allow_low_precision(reason="bf16 softmax denominators"):
                        nc.vector.reciprocal(out=dal[:], in_=dal[:])
                    nc.sync.dma_start(
                        out=bass.AP(tensor=dsi_d.ap().tensor, offset=2 * hp * NQ,
                                    ap=[[2 * NQ // 128, 128], [1, 2 * NQ // 128]]),
                        in_=dal[:])

                def load_bc(e):
                    def f():
                        h = 2 * hp + e
                        den_bc = W.tile([64, NQ], BF16, tag="denb", bufs=2,
                                        name=f"denb{h}")
                        dbs[e] = den_bc
                        nc.sync.dma_start(
                            out=den_bc[:],
                            in_=bass.AP(tensor=dsi_d.ap().tensor, offset=h * NQ,
                                        ap=[[0, 64], [1, NQ]]))
                    return f

                def mul_half(e, half):
                    def f():
                        sl = slice(half * 512, (half + 1) * 512)
                        if e == 0:
                            nc.vector.tensor_mul(out=pairT[hp][0:64, sl],
                                                 in0=numT[h0][0:64, sl],
                                                 in1=dbs[0][:, sl])
                        else:
                            nc.vector.tensor_mul(out=numT[h1][0:64, sl],
                                                 in0=numT[h1][0:64, sl],
                                                 in1=dbs[1][:, sl])
                    return f

                def stitch():
                    nc.sync.dma_start(out=pairT[hp][64:128, :],
                                      in_=numT[h1][0:64, :])

                return [save_den(0), save_den(1), p0, p1,
                        load_bc(0), load_bc(1), mul_half(0, 0), mul_half(0, 1),
                        mul_half(1, 0), mul_half(1, 1), stitch]

            pend = []
            it_idx = 0
            for hp in range(4):
                h0, h1 = 2 * hp, 2 * hp + 1
                for jt in range(NT):
                    for ic in range(2):
                        i5 = ic * 512
                        sp = ps.tile([128, NQ], F32, tag="sp")
                        nc.tensor.matmul(
                            sp[:, 0:512],
                            KTp[hp][0:64, jt * 128:(jt + 1) * 128],
                            QTp[hp][0:64, i5:i5 + 512],
                            start=True, stop=True, tile_position=(0, 0))
                        nc.tensor.matmul(
                            sp[:, 512:1024],
                            KTp[hp][64:128, jt * 128:(jt + 1) * 128],
                            QTp[hp][64:128, i5:i5 + 512],
                            start=True, stop=True, tile_position=(64, 0))
                        eb = W.tile([128, NQ], BF16, tag="eb", bufs=4)
                        nc.scalar.activation(out=eb[:], in_=sp[:], func=AF.Exp)
                        pb = W.tile([128, NQ], BF16, tag="pb", bufs=4)
                        ea = expA[jt][:, i5:i5 + 512].unsqueeze(1) \
                            .broadcast_to([128, 2, 512])
                        nc.vector.tensor_tensor(
                            out=pb[:].rearrange("p (a b) -> p a b", a=2),
                            in0=eb[:].rearrange("p (a b) -> p a b", a=2),
                            in1=ea, op=OP.mult)
                        it_idx += 1
                        nc.tensor.matmul(av0[:, i5:i5 + 512],
                                         Vau[jt][:, h0, 0:65], pb[:, 0:512],
                                         start=(jt == 0), stop=(jt == NT - 1))
                        nc.tensor.matmul(av1[:, i5:i5 + 512],
                                         Vau[jt][:, h1, 0:65], pb[:, 512:1024],
                                         start=(jt == 0), stop=(jt == NT - 1))
                    if prod_by_hp[hp]:
                        prod_by_hp[hp].pop(0)()
                    while pend:
                        f = pend.pop(0)
                        if f is not None:
                            f()
                            break
                for f in pend:
                    if f is not None:
                        f()
                nc.vector.tensor_copy(out=numT[h0][:], in_=av0[:])
                nc.scalar.copy(out=numT[h1][:], in_=av1[:])
                pend = den_pieces(hp)
            for f in pend:
                if f is not None:
                    f()

        if dbg:
            dbg_xnT = nc.dram_tensor("dbg_xnT", [128, 4 * N], BF16,
                                     kind="ExternalOutput")
            nc.sync.dma_start(out=dbg_xnT[:, :],
                              in_=xnT[:].rearrange("p a b -> p (a b)"))
            dbg_QT = nc.dram_tensor("dbg_QT", [128, NQ], BF16,
                                    kind="ExternalOutput")
            nc.sync.dma_start(out=dbg_QT[:, :], in_=QTp[0][:])
            dbg_KT = nc.dram_tensor("dbg_KT", [128, N], BF16,
                                    kind="ExternalOutput")
            nc.sync.dma_start(out=dbg_KT[:, :], in_=KTp[0][:])
            dbg_V = nc.dram_tensor("dbg_V", [128, H * 66], BF16,
                                   kind="ExternalOutput")
            nc.sync.dma_start(out=dbg_V[:, :],
                              in_=Vau[0][:].rearrange("p a b -> p (a b)"))
            dbg_nT = nc.dram_tensor("dbg_nT", [65, NQ], BF16,
                                    kind="ExternalOutput")
            nc.sync.dma_start(out=dbg_nT[:, :], in_=numT[0][:])
            dbg_pT = nc.dram_tensor("dbg_pT", [128, NQ], BF16,
                                    kind="ExternalOutput")
            nc.sync.dma_start(out=dbg_pT[:, :], in_=pairT[0][:])

        # ---- output projection ---------------------------------------------
        with tc.tile_pool(name="psD", bufs=1, space="PSUM") as psD:
            for isl in range(8):
                yp = psD.tile([128, D], F32, tag="yp", bufs=4, name=f"yp{isl}")
                for hp in range(4):
                    nc.tensor.matmul(yp[:],
                                     pairT[hp][:, isl * 128:(isl + 1) * 128],
                                     woutP[hp][:],
                                     start=(hp == 0), stop=(hp == 3))
                ysb = W.tile([128, D], F32, tag="ysb", bufs=4)
                if has_b:
                    nc.vector.tensor_add(out=ysb[:], in0=yp[:], in1=bout_bc[:])
                elif isl % 2 == 0:
                    nc.vector.tensor_copy(out=ysb[:], in_=yp[:])
                else:
                    nc.scalar.copy(out=ysb[:], in_=yp[:])
                eng = nc.sync if isl % 2 == 0 else nc.scalar
                eng.dma_start(out=y_d[isl * 128:(isl + 1) * 128, :],
                              in_=ysb[:])
    if split:
        _split_waits(nc)
    return nc


_NC_CACHE = {}


def _get_nc(has_c, has_b):
    key = (has_c, has_b)
    if key not in _NC_CACHE:
        _NC_CACHE[key] = build(has_c, has_b)
    return _NC_CACHE[key]


LAST_EXEC_TIME_NS = None


def kernel(x, gamma, beta, Wqkv, Wout, bout, rel_table, temporal_mask,
           trace=True):
    global LAST_EXEC_TIME_NS
    x = np.asarray(x, np.float32)
    gamma = np.asarray(gamma, np.float32)
    beta = np.asarray(beta, np.float32)
    Wqkv = np.asarray(Wqkv, np.float32)
    Wout = np.asarray(Wout, np.float32)
    bout = np.asarray(bout, np.float32)
    rel_table = np.asarray(rel_table, np.float32)
    temporal_mask = np.asarray(temporal_mask)

    scale = DH ** -0.5
    w_eff = (Wqkv * gamma[:, None]).copy()
    w_eff[:, :D] *= scale
    cqkv = (beta @ Wqkv).astype(np.float32)
    cqkv[:D] *= scale
    wqkv_bf = w_eff.astype(ml_dtypes.bfloat16)
    wout_bf = Wout.astype(ml_dtypes.bfloat16)
    mask01T = (temporal_mask != 0).T  # [key j, query i]

    exp_table = np.exp(rel_table).astype(np.float32)
    keyperm_half = [
        np.concatenate([np.arange(i0, i0 + NQ),
                        np.arange(NQ - i0, NQ - i0 + NQ)])
        for i0 in (0, NQ)
    ]
    expa_half = []
    for half in range(2):
        kp = keyperm_half[half]
        d = (half * NQ + np.arange(NQ))[None, :] - kp[:, None]
        idx = np.clip(d, -(MAXREL - 1), MAXREL - 1) + MAXREL - 1
        ea = exp_table[idx] * mask01T[kp][:, half * NQ:(half + 1) * NQ]
        expa_half.append(ea.astype(ml_dtypes.bfloat16))

    in_maps = []
    for c in range(NCORES):
        b, half = c // 2, c % 2
        xp = np.ascontiguousarray(
            x[b][keyperm_half[half]]).astype(ml_dtypes.bfloat16)
        in_maps.append({
            "x": xp,
            "wqkv": wqkv_bf,
            "cqkv": cqkv,
            "wout": wout_bf,
            "bout": bout,
            "expa": expa_half[half],
        })

    nc = _get_nc(bool(np.any(cqkv != 0.0)), bool(np.any(bout != 0.0)))
    res = run_bass_kernel_spmd(nc, in_maps, core_ids=list(range(NCORES)),
                               trace=trace)
    LAST_EXEC_TIME_NS = res.exec_time_ns

    out = np.empty((B, N, D), np.float32)
    for c in range(NCORES):
        b, half = c // 2, c % 2
        out[b, half * NQ:(half + 1) * NQ] = res.results[c]["y"]
    return out


# revision 25
# speedup vs baseline: 1.5892x; 1.0260x over previous
"""Trainium2 Bass kernel for nn_Attention_8323646620215.

LayerNorm -> QKV -> scores(+rel-bias+mask) -> softmax -> attn@V -> out proj.

Sharding: 8 cores = (batch b in 0..3) x (query-half in 0..1). Each core
computes the full K/V for its batch and attention for its 1024 query rows;
no cross-core communication.

Fully software-pipelined single-pass schedule:
  - x arrives as bf16 (host-cast), LN on DVE, xn transposed via the DMA
    XBAR (dma_start_transpose) straight into xnT -- no PE transposes.
  - exp(rel-bias)*mask is precomputed on the host and DMA'd as bf16
    [N, NQ] tiles (expA); the mask enters multiplicatively:
    P = exp(S) * expA.
  - QKV production (PE matmuls + psum->SBUF copies) is interleaved into
    the attention head-pair passes so the PE never drains.
  - Per head-pair pass: S_T = K_T^T Q_T (two 64-row PE tiles), ACT exp,
    DVE/Pool multiply by expA, AV accumulate with an augmented ones-row
    for the softmax denominators.
  - Denominators: broadcast-DMA the den row across 64 partitions, then a
    single tensor_tensor divide per head (no DRAM round trip).
  - Output projection immediately follows the last pass.

LN gamma / attention scale / beta are folded into the QKV weights
host-side. The host permutes each core's tokens so its own query half
comes first, keeping the device program identical across cores.
"""
import sys
import types
import numpy as np

sys.path.insert(0, "/opt/trn_rl_repo")

# ---- environment fixes (axon agent container) -------------------------------
if "antenv.axon_hooks" not in sys.modules:
    _m = types.ModuleType("antenv.axon_hooks")
    _m._hook = None
    _m.set_axon_ntff_profile_hook = lambda h: setattr(_m, "_hook", h)
    _m.get_axon_ntff_profile_hook = lambda: _m._hook
    sys.modules["antenv.axon_hooks"] = _m
    try:
        from trn_agent_boot.trn_boot import _ntff_profile_via_ctypes
        _m._hook = _ntff_profile_via_ctypes("/opt/axon/libaxon_pjrt.so")
    except Exception:
        pass

import ml_dtypes  # noqa: E402
from concourse import bass, mybir, tile  # noqa: E402
from concourse.bass_utils import run_bass_kernel_spmd  # noqa: E402

F32 = mybir.dt.float32
BF16 = mybir.dt.bfloat16
AF = mybir.ActivationFunctionType
OP = mybir.AluOpType

B, N, D, H, DH, MAXREL = 4, 2048, 512, 8, 64, 200
NQ = N // 2          # queries per core
NT = N // 128        # 16 token tiles
NCORES = 8

# This container's walrus rejects instructions with more than one sem wait.
# Splitting is sound: a same-engine NoOp right before the instruction
# enforces the wait at the same program point (sequencers run in order).


def _split_waits(nc, maxw=1):
    n_split = 0
    for f in nc.m.functions:
        for blk in f.blocks:
            bb = blk.bb if hasattr(blk, "bb") else blk
            insts = list(bb.instructions)
            out = []
            changed = False
            for inst in insts:
                si = inst.sync_info
                waits = list(si.on_wait) if si and si.on_wait else []
                if len(waits) > maxw:
                    extra = waits[:-maxw]
                    chunks = [extra[j:j + maxw] for j in range(0, len(extra), maxw)]
                    for i, chunk in enumerate(chunks):
                        nop = mybir.InstNoOp(name=f"{inst.name}-ws{i}", ins=[], outs=[])
                        nop.engine = inst.engine
                        nop.sync_info = mybir.SyncInfo(on_wait=chunk, on_update=[])
                        out.append(nop)
                    si.on_wait = waits[-maxw:]
                    changed = True
                    n_split += 1
                out.append(inst)
            if changed:
                bb.instructions = out
    return n_split


def build(has_c=False, has_b=False, split=True, dbg=False):
    nc = bass.Bass("TRN2", target_bir_lowering=False, debug=False,
                   num_devices=NCORES)
    x_d = nc.dram_tensor("x", [N, D], BF16, kind="ExternalInput")
    wqkv_d = nc.dram_tensor("wqkv", [D, 3 * D], BF16, kind="ExternalInput")
    cqkv_d = nc.dram_tensor("cqkv", [3 * D], F32, kind="ExternalInput")
    wout_d = nc.dram_tensor("wout", [D, D], BF16, kind="ExternalInput")
    bout_d = nc.dram_tensor("bout", [D], F32, kind="ExternalInput")
    expa_d = nc.dram_tensor("expa", [N, NQ], BF16, kind="ExternalInput")
    dsb_d = nc.dram_tensor("den_scratch", [H, NQ], BF16)
    dsi_d = nc.dram_tensor("invden_scratch", [H, NQ], BF16)
    y_d = nc.dram_tensor("y", [NQ, D], F32, kind="ExternalOutput")

    with tile.TileContext(nc) as tc, \
         tc.tile_pool(name="const", bufs=1) as C, \
         tc.tile_pool(name="pers", bufs=1) as P, \
         tc.tile_pool(name="work", bufs=3) as W:

        # ---- persistent tiles ----------------------------------------------
        xnT = P.tile([128, 4, N], BF16, tag="xnT", name="xnT")
        KTp = [P.tile([128, N], BF16, tag=f"KT{hp}", name=f"KT{hp}") for hp in range(4)]
        QTp = [P.tile([128, NQ], BF16, tag=f"QT{hp}", name=f"QT{hp}") for hp in range(4)]
        Vau = [P.tile([128, H, 66], BF16, tag=f"V{t}", name=f"Vau{t}") for t in range(NT)]
        expA = [P.tile([128, NQ], BF16, tag=f"eA{t}", name=f"eA{t}") for t in range(NT)]
        numT = [P.tile([65, NQ], BF16, tag=f"nT{h}", name=f"nT{h}") for h in range(H)]
        pairT = [P.tile([128, NQ], BF16, tag=f"pT{hp}", name=f"pT{hp}") for hp in range(4)]

        wqkv_sb = [C.tile([128, 3 * D], BF16, tag=f"wqkv{kb}", name=f"wqkv{kb}") for kb in range(4)]
        woutP = [C.tile([128, D], BF16, tag=f"woutP{hp}", name=f"woutP{hp}") for hp in range(4)]

        # ---- input DMAs -----------------------------------------------------
        # sync (SP HWDGE): x tiles then expA tiles (+ later den broadcasts, y).
        # scalar (ACT HWDGE): XBAR transposes of xn (issued inline in LN flow).
        # gpsimd (SWDGE): weights + small constants.
        x_ts = [W.tile([128, D], BF16, tag="x", bufs=8, name=f"x{t}") for t in range(NT)]
        for t in range(NT):
            nc.sync.dma_start(out=x_ts[t][:], in_=x_d[t * 128:(t + 1) * 128, :])
        for kb in range(4):
            nc.gpsimd.dma_start(out=wqkv_sb[kb][:],
                                in_=wqkv_d[kb * 128:(kb + 1) * 128, :])
        for t in range(NT):
            nc.gpsimd.dma_start(out=expA[t][:],
                                in_=expa_d[t * 128:(t + 1) * 128, :])
        for hp in range(4):
            nc.gpsimd.dma_start(out=woutP[hp][:],
                                in_=wout_d[hp * 128:(hp + 1) * 128, :])
        cq_all = C.tile([128, 12], F32, tag="cq")
        cv_bc = C.tile([128, D], F32, tag="cv")
        bout_bc = C.tile([128, D], F32, tag="bout")
        if has_c:
            nc.gpsimd.dma_start(
                out=cq_all[:],
                in_=bass.AP(tensor=cqkv_d.ap().tensor, offset=0,
                            ap=[[1, 128], [128, 12]]))
            nc.gpsimd.dma_start(
                out=cv_bc[:],
                in_=bass.AP(tensor=cqkv_d.ap().tensor, offset=2 * D,
                            ap=[[0, 128], [1, D]]))
        if has_b:
            nc.gpsimd.dma_start(
                out=bout_bc[:],
                in_=bass.AP(tensor=bout_d.ap().tensor, offset=0,
                            ap=[[0, 128], [1, D]]))

        eps_t = C.tile([128, 1], F32, tag="eps")
        nc.vector.memset(eps_t[:], 1e-5)
        for t in range(NT):
            nc.gpsimd.memset(Vau[t][:, :, 64:66], 1.0)

        with tc.tile_pool(name="ps", bufs=2, space="PSUM") as ps:
            # ---- LayerNorm + XBAR transpose (prologue) ---------------------
            # Staged per group of 4 tiles: stats+sqrt first (DVE+ACT,
            # independent), then apply+transpose (DVE + ACT-queue xbar) so
            # the xbar issues never gate the next tile's sqrt.
            mvs = [None] * NT
            rss = [None] * NT

            def ln_stats(t):
                st = W.tile([128, 6], F32, tag="st")
                nc.vector.bn_stats(out=st[:], in_=x_ts[t][:])
                mv = W.tile([128, 2], F32, tag="mv", bufs=8)
                nc.vector.bn_aggr(out=mv[:], in_=st[:])
                rs = W.tile([128, 1], F32, tag="rs", bufs=8)
                nc.scalar.activation(out=rs[:], in_=mv[:, 1:2], func=AF.Sqrt,
                                     bias=eps_t[:])
                mvs[t], rss[t] = mv, rs

            def ln_apply(t):
                nc.vector.reciprocal(out=rss[t][:], in_=rss[t][:])
                xn = W.tile([128, D], BF16, tag="xn", bufs=3)
                nc.vector.tensor_scalar(out=xn[:], in0=x_ts[t][:],
                                        scalar1=mvs[t][:, 0:1],
                                        scalar2=rss[t][:],
                                        op0=OP.subtract, op1=OP.mult)
                nc.scalar.dma_start_transpose(
                    xnT[:, :, t * 128:(t + 1) * 128], xn[:])

            def ln_tile(t):
                ln_stats(t)
                ln_apply(t)

            # ---- QKV production tasks (PE + copy), interleaved -------------
            def q_task(hp):
                def f():
                    qp = ps.tile([128, NQ], F32, tag="sp")
                    for ic in range(2):
                        for kb in range(4):
                            nc.tensor.matmul(
                                qp[:, ic * 512:(ic + 1) * 512],
                                wqkv_sb[kb][:, hp * 128:(hp + 1) * 128],
                                xnT[:, kb, ic * 512:(ic + 1) * 512],
                                start=(kb == 0), stop=(kb == 3))
                    if has_c:
                        nc.vector.tensor_scalar_add(
                            out=QTp[hp][:], in0=qp[:],
                            scalar1=cq_all[:, hp:hp + 1])
                    else:
                        nc.scalar.copy(out=QTp[hp][:], in_=qp[:])
                return f

            def k_task(hp, half):
                def f():
                    kp = ps.tile([128, NQ], F32, tag="sp")
                    for tc4 in range(2):
                        for kb in range(4):
                            nc.tensor.matmul(
                                kp[:, tc4 * 512:(tc4 + 1) * 512],
                                wqkv_sb[kb][:, D + hp * 128:D + (hp + 1) * 128],
                                xnT[:, kb,
                                    half * 1024 + tc4 * 512:
                                    half * 1024 + (tc4 + 1) * 512],
                                start=(kb == 0), stop=(kb == 3))
                    if has_c:
                        nc.vector.tensor_scalar_add(
                            out=KTp[hp][:, half * 1024:(half + 1) * 1024],
                            in0=kp[:], scalar1=cq_all[:, 4 + hp:5 + hp])
                    else:
                        nc.scalar.copy(
                            out=KTp[hp][:, half * 1024:(half + 1) * 1024],
                            in_=kp[:])
                return f

            def v_task(j2):
                def f():
                    vp = ps.tile([128, NQ], F32, tag="sp")
                    for e in range(2):
                        jt = 2 * j2 + e
                        for kb in range(4):
                            nc.tensor.matmul(
                                vp[:, e * 512:(e + 1) * 512],
                                xnT[:, kb, jt * 128:(jt + 1) * 128],
                                wqkv_sb[kb][:, 2 * D:3 * D],
                                start=(kb == 0), stop=(kb == 3))
                    for e in range(2):
                        jt = 2 * j2 + e
                        src = vp[:, e * 512:(e + 1) * 512].rearrange(
                            "p (h d) -> p h d", h=H)
                        if has_c:
                            nc.vector.tensor_add(
                                out=Vau[jt][:, :, 0:64], in0=src,
                                in1=cv_bc[:].rearrange("p (h d) -> p h d", h=H))
                        else:
                            nc.scalar.copy(out=Vau[jt][:, :, 0:64], in_=src)
                return f

            # Emit LN for the query half, then enough QKV to start attention.
            for t in range(4):
                ln_stats(t)
            for t in range(4):
                ln_apply(t)
            for t in range(4, 8):
                ln_stats(t)
            for t in range(4, 8):
                ln_apply(t)
            q_task(0)()
            k_task(0, 0)()
            v_task(0)()
            v_task(1)()

            def ln_task(t):
                def f():
                    ln_stats(t)
                    ln_apply(t)
                return f

            # prod tasks popped one per jt during passes; V(j2) must land
            # before jt=2*j2 of hp0, K/Q for pass hp before that pass starts.
            prod_sched = {
                (0, 0): [ln_task(8), ln_task(9)],
                (0, 1): [v_task(2), ln_task(10)],
                (0, 2): [ln_task(11), v_task(3)],
                (0, 3): [ln_task(12), ln_task(13)],
                (0, 4): [v_task(4), ln_task(14)],
                (0, 5): [ln_task(15), v_task(5)],
                (0, 6): [k_task(0, 1), v_task(6)],
                (0, 7): [v_task(7), q_task(1)],
                (0, 8): [k_task(1, 0)],
                (1, 0): [k_task(1, 1)], (1, 2): [q_task(2)],
                (1, 4): [k_task(2, 0)],
                (2, 0): [k_task(2, 1)], (2, 2): [q_task(3)],
                (2, 4): [k_task(3, 0)], (2, 6): [k_task(3, 1)],
            }

            # ---- attention passes ------------------------------------------
            av0 = ps.tile([65, NQ], F32, tag="av0", bufs=1, name="av0")
            av1 = ps.tile([65, NQ], F32, tag="av1", bufs=1, name="av1")

            def den_pieces(hp):
                """Normalize pair hp's heads; interleaved into next pass.
                Denominator rows bounce via DRAM into a [128,16] tile so the
                reciprocal runs wide, then broadcast across 64 partitions."""
                h0, h1 = 2 * hp, 2 * hp + 1
                dal = W.tile([128, 2 * NQ // 128], BF16, tag="dall", bufs=2,
                             name=f"dal{hp}")
                dbs = [None, None]

                def save_den(e):
                    def f():
                        h = 2 * hp + e
                        nc.sync.dma_start(out=dsb_d[h, :],
                                          in_=numT[h][64:65, :])
                    return f

                def p0():
                    nc.sync.dma_start(
                        out=dal[:],
                        in_=bass.AP(tensor=dsb_d.ap().tensor, offset=2 * hp * NQ,
                                    ap=[[2 * NQ // 128, 128], [1, 2 * NQ // 128]]))

                def p1():
                    with nc.allow_low_precision(reason="bf16 softmax denominators"):
                        nc.vector.reciprocal(out=dal[:], in_=dal[:])
                    nc.sync.dma_start(
                        out=bass.AP(tensor=dsi_d.ap().tensor, offset=2 * hp * NQ,
                                    ap=[[2 * NQ // 128, 128], [1, 2 * NQ // 128]]),
                        in_=dal[:])

                def load_bc(e):
                    def f():
                        h = 2 * hp + e
                        den_bc = W.tile([64, NQ], BF16, tag="denb", bufs=2,
                                        name=f"denb{h}")
                        dbs[e] = den_bc
                        nc.sync.dma_start(
                            out=den_bc[:],
                            in_=bass.AP(tensor=dsi_d.ap().tensor, offset=h * NQ,
                                        ap=[[0, 64], [1, NQ]]))
                    return f

                def mul_half(e, half):
                    def f():
                        sl = slice(half * 512, (half + 1) * 512)
                        if e == 0:
                            nc.vector.tensor_mul(out=pairT[hp][0:64, sl],
                                                 in0=numT[h0][0:64, sl],
                                                 in1=dbs[0][:, sl])
                        else:
                            nc.vector.tensor_mul(out=numT[h1][0:64, sl],
                                                 in0=numT[h1][0:64, sl],
                                                 in1=dbs[1][:, sl])
                    return f

                def stitch():
                    nc.sync.dma_start(out=pairT[hp][64:128, :],
                                      in_=numT[h1][0:64, :])

                return [save_den(0), save_den(1), p0, p1,
                        load_bc(0), load_bc(1), mul_half(0, 0), mul_half(0, 1),
                        mul_half(1, 0), mul_half(1, 1), stitch]

            pend = []
            it_idx = 0
            for hp in range(4):
                h0, h1 = 2 * hp, 2 * hp + 1
                for jt in range(NT):
                    for ic in range(2):
                        i5 = ic * 512
                        sp = ps.tile([128, NQ], F32, tag="sp")
                        nc.tensor.matmul(
                            sp[:, 0:512],
                            KTp[hp][0:64, jt * 128:(jt + 1) * 128],
                            QTp[hp][0:64, i5:i5 + 512],
                            start=True, stop=True, tile_position=(0, 0))
                        nc.tensor.matmul(
                            sp[:, 512:1024],
                            KTp[hp][64:128, jt * 128:(jt + 1) * 128],
                            QTp[hp][64:128, i5:i5 + 512],
                            start=True, stop=True, tile_position=(64, 0))
                        eb = W.tile([128, NQ], BF16, tag="eb", bufs=4)
                        nc.scalar.activation(out=eb[:], in_=sp[:], func=AF.Exp)
                        pb = W.tile([128, NQ], BF16, tag="pb", bufs=4)
                        ea = expA[jt][:, i5:i5 + 512].unsqueeze(1) \
                            .broadcast_to([128, 2, 512])
                        nc.vector.tensor_tensor(
                            out=pb[:].rearrange("p (a b) -> p a b", a=2),
                            in0=eb[:].rearrange("p (a b) -> p a b", a=2),
                            in1=ea, op=OP.mult)
                        it_idx += 1
                        nc.tensor.matmul(av0[:, i5:i5 + 512],
                                         Vau[jt][:, h0, 0:65], pb[:, 0:512],
                                         start=(jt == 0), stop=(jt == NT - 1))
                        nc.tensor.matmul(av1[:, i5:i5 + 512],
                                         Vau[jt][:, h1, 0:65], pb[:, 512:1024],
                                         start=(jt == 0), stop=(jt == NT - 1))
                    for task in prod_sched.get((hp, jt), []):
                        task()
                    while pend:
                        f = pend.pop(0)
                        if f is not None:
                            f()
                            break
                for f in pend:
                    if f is not None:
                        f()
                nc.vector.tensor_copy(out=numT[h0][:], in_=av0[:])
                nc.scalar.copy(out=numT[h1][:], in_=av1[:])
                pend = den_pieces(hp)
            for f in pend:
                if f is not None:
                    f()

        if dbg:
            dbg_xnT = nc.dram_tensor("dbg_xnT", [128, 4 * N], BF16,
                                     kind="ExternalOutput")
            nc.sync.dma_start(out=dbg_xnT[:, :],
                              in_=xnT[:].rearrange("p a b -> p (a b)"))
            dbg_QT = nc.dram_tensor("dbg_QT", [128, NQ], BF16,
                                    kind="ExternalOutput")
            nc.sync.dma_start(out=dbg_QT[:, :], in_=QTp[0][:])
            dbg_KT = nc.dram_tensor("dbg_KT", [128, N], BF16,
                                    kind="ExternalOutput")
            nc.sync.dma_start(out=dbg_KT[:, :], in_=KTp[0][:])
            dbg_V = nc.dram_tensor("dbg_V", [128, H * 66], BF16,
                                   kind="ExternalOutput")
            nc.sync.dma_start(out=dbg_V[:, :],
                              in_=Vau[0][:].rearrange("p a b -> p (a b)"))
            dbg_nT = nc.dram_tensor("dbg_nT", [65, NQ], BF16,
                                    kind="ExternalOutput")
            nc.sync.dma_start(out=dbg_nT[:, :], in_=numT[0][:])
            dbg_pT = nc.dram_tensor("dbg_pT", [128, NQ], BF16,
                                    kind="ExternalOutput")
            nc.sync.dma_start(out=dbg_pT[:, :], in_=pairT[0][:])

        # ---- output projection ---------------------------------------------
        with tc.tile_pool(name="psD", bufs=1, space="PSUM") as psD:
            for isl in range(8):
                yp = psD.tile([128, D], F32, tag="yp", bufs=4, name=f"yp{isl}")
                for hp in range(4):
                    nc.tensor.matmul(yp[:],
                                     pairT[hp][:, isl * 128:(isl + 1) * 128],
                                     woutP[hp][:],
                                     start=(hp == 0), stop=(hp == 3))
                ysb = W.tile([128, D], F32, tag="ysb", bufs=4)
                if has_b:
                    nc.vector.tensor_add(out=ysb[:], in0=yp[:], in1=bout_bc[:])
                elif isl % 2 == 0:
                    nc.vector.tensor_copy(out=ysb[:], in_=yp[:])
                else:
                    nc.scalar.copy(out=ysb[:], in_=yp[:])
                eng = nc.sync if isl % 2 == 0 else nc.scalar
                eng.dma_start(out=y_d[isl * 128:(isl + 1) * 128, :],
                              in_=ysb[:])
    if split:
        _split_waits(nc)
    return nc


_NC_CACHE = {}


def _get_nc(has_c, has_b):
    key = (has_c, has_b)
    if key not in _NC_CACHE:
        _NC_CACHE[key] = build(has_c, has_b)
    return _NC_CACHE[key]


LAST_EXEC_TIME_NS = None


def kernel(x, gamma, beta, Wqkv, Wout, bout, rel_table, temporal_mask,
           trace=True):
    global LAST_EXEC_TIME_NS
    x = np.asarray(x, np.float32)
    gamma = np.asarray(gamma, np.float32)
    beta = np.asarray(beta, np.float32)
    Wqkv = np.asarray(Wqkv, np.float32)
    Wout = np.asarray(Wout, np.float32)
    bout = np.asarray(bout, np.float32)
    rel_table = np.asarray(rel_table, np.float32)
    temporal_mask = np.asarray(temporal_mask)

    scale = DH ** -0.5
    w_eff = (Wqkv * gamma[:, None]).copy()
    w_eff[:, :D] *= scale
    cqkv = (beta @ Wqkv).astype(np.float32)
    cqkv[:D] *= scale
    wqkv_bf = w_eff.astype(ml_dtypes.bfloat16)
    wout_bf = Wout.astype(ml_dtypes.bfloat16)
    mask01T = (temporal_mask != 0).T  # [key j, query i]

    exp_table = np.exp(rel_table).astype(np.float32)
    keyperm_half = [
        np.concatenate([np.arange(i0, i0 + NQ),
                        np.arange(NQ - i0, NQ - i0 + NQ)])
        for i0 in (0, NQ)
    ]
    expa_half = []
    for half in range(2):
        kp = keyperm_half[half]
        d = (half * NQ + np.arange(NQ))[None, :] - kp[:, None]
        idx = np.clip(d, -(MAXREL - 1), MAXREL - 1) + MAXREL - 1
        ea = exp_table[idx] * mask01T[kp][:, half * NQ:(half + 1) * NQ]
        expa_half.append(ea.astype(ml_dtypes.bfloat16))

    in_maps = []
    for c in range(NCORES):
        b, half = c // 2, c % 2
        xp = np.ascontiguousarray(
            x[b][keyperm_half[half]]).astype(ml_dtypes.bfloat16)
        in_maps.append({
            "x": xp,
            "wqkv": wqkv_bf,
            "cqkv": cqkv,
            "wout": wout_bf,
            "bout": bout,
            "expa": expa_half[half],
        })

    nc = _get_nc(bool(np.any(cqkv != 0.0)), bool(np.any(bout != 0.0)))
    res = run_bass_kernel_spmd(nc, in_maps, core_ids=list(range(NCORES)),
                               trace=trace)
    LAST_EXEC_TIME_NS = res.exec_time_ns

    out = np.empty((B, N, D), np.float32)
    for c in range(NCORES):
        b, half = c // 2, c % 2
        out[b, half * NQ:(half + 1) * NQ] = res.results[c]["y"]
    return out
